# revision 19
# baseline (speedup 1.0000x reference)
"""BevFeatureEncoder on 8 Trainium2 NeuronCores.

Strategy (data-parallel over BEV grid slabs):
  - The 2*480*360 BEV cells are split into 8 contiguous ranges of 43200
    cells; points are routed on host to the core owning their cell, so
    the segment_max reduction is fully local to each core.
  - Per core, occupied cells are sorted by point count DESC and grouped
    into chunks of 2048 cells. Slot s of a chunk covers only the prefix
    of cells with count > s (widths shrink with s, rounded to 256), so
    there is no power-of-2 slot padding. Items run in s-major order so
    the per-chunk max-accumulate chains never serialize back-to-back.
  - BN scale/shift is folded into the weights ON HOST (numpy): the
    device sees pre-folded bf16 stationaries + per-partition biases and
    has no weight-prep preamble.
  - Device dataflow: 2048-wide units, each 4 matmuls into a 4-bank PSUM
    tile + ONE wide drain (relu+bias) on ACT or DVE, debt-balanced.
    Layer 1 packs two cells per column (block-diag [8,128]); layer 2
    unpacks via even/odd zero-padded stationaries; the compression runs
    col-tiled (out partitions 0:64 / 64:128 for a chunk pair). Max
    accumulation (slots s>0) is split: ACT/DVE relu-drain to a temp
    tile, then a 2x-rate bf16 SBUF tensor_tensor max into the
    accumulator planes.
  - Output is compacted [64, G*2048] bf16; host places real columns
    into the zeroed [B, C, GX, GY] grid. Chunk structure is equalized
    across cores (max widths) so one SPMD program serves all 8 cores.
"""

import numpy as np

import concourse.bacc as bacc
import concourse.bass as bass
import concourse.mybir as mybir
import concourse.tile as tile
from concourse import bass_utils

GX, GY = 480, 360
B = 2
EPS = 1e-5
N_CORES = 8
CELLS_PER_CORE = (B * GX * GY) // N_CORES  # 43200
CHUNK = 2048  # cells per chunk (fin granularity; 4 PSUM banks wide)
PAD = -1

F32 = mybir.dt.float32
BF16 = mybir.dt.bfloat16

Relu = mybir.ActivationFunctionType.Relu


# ---------------------------------------------------------------- host prep


def _build_plan_and_data(voxels, coors):
    """Route points to cores; build the equalized slot plan and per-core
    device inputs."""
    seg = (
        coors[:, 0].astype(np.int64) * (GX * GY)
        + coors[:, 1].astype(np.int64) * GY
        + coors[:, 2].astype(np.int64)
    )
    core_of = seg // CELLS_PER_CORE

    per_core = []
    for c in range(N_CORES):
        idx = np.nonzero(core_of == c)[0]
        seg_local = seg[idx] - c * CELLS_PER_CORE
        order = np.argsort(seg_local, kind="stable")
        seg_sorted = seg_local[order]
        cells, starts, counts = np.unique(
            seg_sorted, return_index=True, return_counts=True
        )
        o2 = np.argsort(-counts, kind="stable")
        cells, starts, counts = cells[o2], starts[o2], counts[o2]
        pts = idx[order]
        per_core.append((cells, starts, counts, pts))

    n_occ_max = max(len(pc[0]) for pc in per_core)
    G = -(-n_occ_max // CHUNK)
    if G % 2:
        G += 1  # fin works on chunk pairs
    kmax = int(max(pc[2].max() for pc in per_core))

    widths = np.zeros((kmax, G), np.int64)
    for cells, starts, counts, pts in per_core:
        n = len(cells)
        for g in range(G):
            lo, hi = g * CHUNK, min((g + 1) * CHUNK, n)
            if lo >= hi:
                continue
            cg = counts[lo:hi]
            for s in range(int(cg[0])):
                widths[s, g] = max(widths[s, g], int((cg > s).sum()))
    widths = np.minimum(-(-widths // 256) * 256, CHUNK)

    # items in s-major order; greedy-pack h1 blocks (w//2 cols) into
    # 2048-col groups so every item's h1 is contiguous in one tile
    items = []  # (s, g, w, grp, off)
    grp, off = 0, 0
    for s in range(kmax):
        for g in range(G):
            w = int(widths[s, g])
            if w == 0:
                continue
            h = w // 2
            if off + h > 2048:
                grp += 1
                off = 0
            items.append((s, g, w, grp, off))
            off += h
    G1 = grp + 1

    vox_all = np.zeros((N_CORES, 8, G1 * 2048), np.float32)
    rows_all = np.full((N_CORES, G * CHUNK), PAD, np.int64)

    for core, (cells, starts, counts, pts) in enumerate(per_core):
        n = len(cells)
        rows_all[core, :n] = cells
        for (s, g, w, gp, of) in items:
            lo = g * CHUNK
            ncell = max(0, min(w, n - lo))
            if ncell == 0:
                continue
            cnt = counts[lo : lo + ncell]
            so = np.minimum(s, cnt - 1)
            p_idx = pts[starts[lo : lo + ncell] + so]
            feats = np.zeros((w, 4), np.float32)
            feats[:ncell] = voxels[p_idx]
            h = w // 2
            packed = np.concatenate([feats[:h].T, feats[h:].T], axis=0)
            pc0 = gp * 2048 + of
            vox_all[core, :, pc0 : pc0 + h] = packed
    return (G, kmax, tuple(widths.flatten().tolist())), items, G1, \
        vox_all, rows_all


def _fold_weights(w1, g1, b1, m1, v1, w2, g2, b2, m2, v2,
                  w3, g3, b3, m3, v3, wc, bc):
    """Fold BN (eval) into the linear weights, build device layouts."""
    import ml_dtypes

    def sb(g, b, m, v):
        s = g / np.sqrt(v + EPS)
        return s.astype(np.float32), (b - m * s).astype(np.float32)

    s1, t1 = sb(g1, b1, m1, v1)
    s2, t2 = sb(g2, b2, m2, v2)
    s3, t3 = sb(g3, b3, m3, v3)
    w1f = (w1 * s1).astype(np.float32)  # [4,64]
    w2f = (w2 * s2).astype(np.float32)  # [64,128]
    w3f = (w3 * s3).astype(np.float32)  # [128,256]

    bf = ml_dtypes.bfloat16
    w1d8 = np.zeros((8, 128), np.float32)
    w1d8[0:4, 0:64] = w1f
    w1d8[4:8, 64:128] = w1f
    w2e = np.zeros((128, 128), np.float32)
    w2e[0:64] = w2f
    w2o = np.zeros((128, 128), np.float32)
    w2o[64:128] = w2f
    out = {
        "w1d8": w1d8.astype(bf),
        "w2e": w2e.astype(bf),
        "w2o": w2o.astype(bf),
        "w3a": np.ascontiguousarray(w3f[:, 0:128]).astype(bf),
        "w3b": np.ascontiguousarray(w3f[:, 128:256]).astype(bf),
        "wc0": np.ascontiguousarray(wc[0:128]).astype(np.float32).astype(bf),
        "wc1": np.ascontiguousarray(wc[128:256]).astype(np.float32).astype(bf),
        "t1d2": np.concatenate([t1, t1])[:, None].astype(np.float32),
        "t2": t2[:, None].astype(np.float32),
        "t3a": t3[0:128, None].astype(np.float32),
        "t3b": t3[128:256, None].astype(np.float32),
        "bc2": np.concatenate([bc, bc])[:, None].astype(np.float32),
    }
    return out


# ------------------------------------------------------------- bass program


def build_program(G, items, G1):
    nc = bacc.Bacc("TRN2", target_bir_lowering=False, debug=False,
                   num_devices=N_CORES)

    vox = nc.dram_tensor("vox", [8, G1 * 2048], BF16,
                         kind="ExternalInput").ap()
    wdram = {}
    for name, shape, dt in [
        ("w1d8", [8, 128], BF16), ("w2e", [128, 128], BF16),
        ("w2o", [128, 128], BF16),
        ("w3a", [128, 128], BF16), ("w3b", [128, 128], BF16),
        ("wc0", [128, 64], BF16), ("wc1", [128, 64], BF16),
        ("t1d2", [128, 1], F32), ("t2", [128, 1], F32),
        ("t3a", [128, 1], F32), ("t3b", [128, 1], F32),
        ("bc2", [128, 1], F32),
    ]:
        wdram[name] = (nc.dram_tensor(name, shape, dt,
                                      kind="ExternalInput").ap(), shape, dt)
    comp = nc.dram_tensor("comp", [64, G * CHUNK], BF16,
                          kind="ExternalOutput").ap()

    from contextlib import ExitStack
    with tile.TileContext(nc) as tc, ExitStack() as ctx:
        cpool = ctx.enter_context(tc.tile_pool(name="const", bufs=1))

        # weight/bias loads: spread over queues; w1-chain first so the
        # first p1 unit can start immediately
        _q = [nc.scalar, nc.gpsimd, nc.sync]
        wt = {}
        for i, name in enumerate(["w1d8", "t1d2", "w2e", "w2o", "t2",
                                  "w3a", "w3b", "t3a", "t3b",
                                  "wc0", "wc1", "bc2"]):
            ap, shape, dt = wdram[name]
            t = cpool.tile(shape, dt, tag=name)
            _q[i % 3].dma_start(out=t[:], in_=ap)
            wt[name] = t

        vox_sb = cpool.tile([8, G1 * 2048], BF16)
        for i in range(3):
            lo = (G1 * 2048 // 3) * i
            hi = (G1 * 2048 // 3) * (i + 1) if i < 2 else G1 * 2048
            _q[i].dma_start(out=vox_sb[:, lo:hi], in_=vox[:, lo:hi])

        accA = cpool.tile([128, G * CHUNK], BF16)
        accB = cpool.tile([128, G * CHUNK], BF16)
        w0 = {}
        for (s, g, w, gp, of) in items:
            w0[g] = max(w0.get(g, 0), w)
        for g in range(G):
            wg = w0.get(g, 0)
            if wg < CHUNK:
                nc.vector.memset(accA[:, g * CHUNK + wg : (g + 1) * CHUNK], 0.0)
                nc.vector.memset(accB[:, g * CHUNK + wg : (g + 1) * CHUNK], 0.0)

        h1p = ctx.enter_context(tc.tile_pool(name="h1p", bufs=4))
        h2p = ctx.enter_context(tc.tile_pool(name="h2p", bufs=5))
        tmp = ctx.enter_context(tc.tile_pool(name="tmp", bufs=4))
        scp = ctx.enter_context(tc.tile_pool(name="scp", bufs=2))
        psum = ctx.enter_context(tc.tile_pool(name="psum", bufs=2,
                                              space="PSUM"))

        # drain routing: debt-balanced over ACT / DVE (constants fit to
        # HW-measured slice durations at N=2048)
        debt = {"act": 0.0, "dve": 0.0}
        COST = {
            "act": lambda n: (n + 352.0) / 1.46,
            "dve": lambda n: (n + 120.0) / 1.13,
        }

        def br_eng(eng, out_ap, in_ap, bias_ap):
            debt[eng] += COST[eng](in_ap.shape[-1])
            if eng == "act":
                nc.scalar.activation(out_ap, in_ap, Relu, bias=bias_ap,
                                     scale=1.0)
            else:
                nc.vector.tensor_scalar(out_ap, in_ap, bias_ap, 0.0,
                                        op0=mybir.AluOpType.add,
                                        op1=mybir.AluOpType.max)

        def br_auto(out_ap, in_ap, bias_ap):
            # split the drain across BOTH engines: halves the PSUM
            # release latency so the PE's next unit starts sooner
            n = in_ap.shape[-1]
            e0 = min(("act", "dve"), key=lambda e: debt[e] + COST[e](n // 2))
            e1 = "dve" if e0 == "act" else "act"
            if n < 1024:
                br_eng(e0, out_ap, in_ap, bias_ap)
                return
            h = (n // 2 + 511) // 512 * 512  # left half at bank boundary
            br_eng(e0, out_ap[:, 0:h], in_ap[:, 0:h], bias_ap)
            br_eng(e1, out_ap[:, h:n], in_ap[:, h:n], bias_ap)

        h1t = {}

        def unit_p1(t):
            p = psum.tile([128, 2048], F32, tag="ps", space="PSUM",
                          name=f"p1_{t}")
            slab = vox_sb[:, t * 2048 : (t + 1) * 2048]
            for i in range(4):
                nc.tensor.matmul(p[:, i * 512 : (i + 1) * 512],
                                 wt["w1d8"][:],
                                 slab[:, i * 512 : (i + 1) * 512],
                                 start=True, stop=True)
            h1 = h1p.tile([128, 2048], BF16, tag="h1", name=f"h1_{t}")
            br_auto(h1[:], p[:], wt["t1d2"][:])
            h1t[t] = h1

        def seg(c0, width):
            # split [c0, c0+width) at 512-col PSUM bank boundaries
            j = c0
            while j < c0 + width:
                j1 = min((j // 512 + 1) * 512, c0 + width)
                yield j, j1
                j = j1

        def unit_p2(it):
            (s, g, w, gp, of) = it
            h = w // 2
            h1s = h1t[gp]
            p = psum.tile([128, 2048], F32, tag="ps", space="PSUM",
                          name=f"p2_{s}_{g}")[:, 0:w]
            for j, j1 in seg(0, h):
                nc.tensor.matmul(p[:, j:j1], wt["w2e"][:],
                                 h1s[:, of + j : of + j1],
                                 start=True, stop=True)
            for j, j1 in seg(h, h):
                nc.tensor.matmul(p[:, j:j1], wt["w2o"][:],
                                 h1s[:, of + j - h : of + j1 - h],
                                 start=True, stop=True)
            h2 = h2p.tile([128, 2048], BF16, tag="h2",
                          name=f"h2_{s}_{g}")[:, 0:w]
            br_auto(h2, p, wt["t2"][:])
            return h2

        def unit_p3(it, h2_ap, half):
            (s, g, w, gp, of) = it
            wst = wt["w3a"] if half == 0 else wt["w3b"]
            bias = wt["t3a"] if half == 0 else wt["t3b"]
            acc = (accA if half == 0 else accB)[:, g * CHUNK : g * CHUNK + w]
            p = psum.tile([128, 2048], F32, tag="ps", space="PSUM",
                          name=f"p3{half}_{s}_{g}")[:, 0:w]
            for j, j1 in seg(0, w):
                nc.tensor.matmul(p[:, j:j1], wst[:], h2_ap[:, j:j1],
                                 start=True, stop=True)
            if s == 0:
                br_auto(acc, p, bias[:])
            else:
                # relu-drain to temp (ACT/DVE), then 2x-rate bf16 SBUF
                # max into acc on DVE (acc >= 0 so relu commutes w/ max)
                tt = tmp.tile([128, 2048], BF16, tag="tt",
                              name=f"tt{half}_{s}_{g}")[:, 0:w]
                br_auto(tt, p, bias[:])
                debt["dve"] += (w / 2.0 + 151.0) / 0.96
                nc.vector.tensor_max(acc, tt, acc)

        def unit_fin(g):
            p = psum.tile([128, 2048], F32, tag="ps", space="PSUM",
                          name=f"pc_{g}")
            # j outer / q inner: adjacent accumulation regions alternate
            # PE column groups (out partitions 0:64 / 64:128) so their
            # matmuls overlap in the array
            for j in range(0, 2048, 512):
                for q in range(2):
                    cols = (g + q) * CHUNK
                    dst = p[64 * q : 64 * q + 64, :]
                    nc.tensor.matmul(dst[:, j : j + 512], wt["wc0"][:],
                                     accA[:, cols + j : cols + j + 512],
                                     start=True, stop=False)
                    nc.tensor.matmul(dst[:, j : j + 512], wt["wc1"][:],
                                     accB[:, cols + j : cols + j + 512],
                                     start=False, stop=True)
            sc = scp.tile([128, 2048], BF16, tag="sc", name=f"sc_{g}")
            br_auto(sc[:], p[:], wt["bc2"][:])
            o = g * CHUNK
            nc.gpsimd.dma_start(out=comp[:, o : o + CHUNK], in_=sc[0:64, :])
            nc.gpsimd.dma_start(out=comp[:, o + CHUNK : o + 2 * CHUNK],
                                in_=sc[64:128, :])

        # ---- software-pipelined emission ----
        last_s = {}
        for (s, g, w, gp, of) in items:
            last_s[g] = s
        done = [False] * G
        finned = [False] * G

        def try_fins(g):
            done[g] = True
            gp = g - (g % 2)
            if done[gp] and done[gp + 1] and not finned[gp]:
                finned[gp] = True
                unit_fin(gp)

        have_items = {g for (_, g, _, _, _) in items}
        for g in range(G):
            if g not in have_items:
                done[g] = True
        for g in range(0, G, 2):
            if done[g] and done[g + 1]:
                finned[g] = True

        p2q = list(items)
        p3q = []

        def pump(grp_done_upto):
            while p3q:
                it, h2a = p3q.pop(0)
                unit_p3(it, h2a, 0)
                unit_p3(it, h2a, 1)
                if it[0] == last_s[it[1]]:
                    try_fins(it[1])
            while p2q and p2q[0][3] < grp_done_upto:
                it = p2q.pop(0)
                p3q.append((it, unit_p2(it)))

        for t in range(G1):
            unit_p1(t)
            pump(t)
        pump(G1)
        pump(G1)

    nc.compile()
    return nc


# ------------------------------------------------------------------ driver

_CACHE = {}


def kernel(voxels, coors, batch_size, w1, g1, b1, m1, v1,
           w2, g2, b2, m2, v2, w3, g3, b3, m3, v3, wc, bc,
           _trace=False):
    voxels = np.asarray(voxels, np.float32)
    coors = np.asarray(coors, np.int32)
    plan_key, items, G1, vox_all, rows_all = _build_plan_and_data(
        voxels, coors)
    G = plan_key[0]

    if plan_key not in _CACHE:
        _CACHE[plan_key] = build_program(G, items, G1)
    nc = _CACHE[plan_key]

    folded = _fold_weights(
        np.asarray(w1, np.float32), np.asarray(g1, np.float32),
        np.asarray(b1, np.float32), np.asarray(m1, np.float32),
        np.asarray(v1, np.float32),
        np.asarray(w2, np.float32), np.asarray(g2, np.float32),
        np.asarray(b2, np.float32), np.asarray(m2, np.float32),
        np.asarray(v2, np.float32),
        np.asarray(w3, np.float32), np.asarray(g3, np.float32),
        np.asarray(b3, np.float32), np.asarray(m3, np.float32),
        np.asarray(v3, np.float32),
        np.asarray(wc, np.float32), np.asarray(bc, np.float32))

    import ml_dtypes
    in_maps = [
        {"vox": vox_all[c].astype(ml_dtypes.bfloat16), **folded}
        for c in range(N_CORES)
    ]
    res = bass_utils.run_bass_kernel_spmd(
        nc, in_maps, core_ids=list(range(N_CORES)), trace=_trace)

    out = np.zeros((B, 64, GX * GY), np.float32)
    for c in range(N_CORES):
        cm = np.asarray(res.results[c]["comp"]).astype(np.float32)
        rows = rows_all[c]
        real = rows != PAD
        gcell = rows[real] + c * CELLS_PER_CORE
        b_core = c // (N_CORES // B)
        xy = gcell - b_core * (GX * GY)
        out[b_core][:, xy] = cm[:, real]
    out = out.reshape(B, 64, GX, GY)
    if _trace:
        return out, res
    return out


# revision 21
# speedup vs baseline: 1.4509x; 1.4509x over previous
"""BevFeatureEncoder on 8 Trainium2 NeuronCores.

Strategy (data-parallel over BEV grid slabs):
  - The 2*480*360 BEV cells are split into 8 contiguous ranges of 43200
    cells; points are routed on host to the core owning their cell, so
    the segment_max reduction is fully local to each core.
  - Per core, occupied cells are sorted by point count DESC and grouped
    into chunks of 2048 cells. Slot s of a chunk covers only the prefix
    of cells with count > s (widths shrink with s, rounded to 256), so
    there is no power-of-2 slot padding. Items run in s-major order so
    the per-chunk max-accumulate chains never serialize back-to-back.
  - BN scale/shift is folded into the weights ON HOST (numpy): the
    device sees pre-folded bf16 stationaries + per-partition biases and
    has no weight-prep preamble.
  - Device dataflow: 2048-wide units, each 4 matmuls into a 4-bank PSUM
    tile + ONE wide drain (relu+bias) on ACT or DVE, debt-balanced.
    Layer 1 packs two cells per column (block-diag [8,128]); layer 2
    unpacks via even/odd zero-padded stationaries; the compression runs
    col-tiled (out partitions 0:64 / 64:128 for a chunk pair). Max
    accumulation (slots s>0) is split: ACT/DVE relu-drain to a temp
    tile, then a 2x-rate bf16 SBUF tensor_tensor max into the
    accumulator planes.
  - Output is compacted [64, G*2048] bf16; host places real columns
    into the zeroed [B, C, GX, GY] grid. Chunk structure is equalized
    across cores (max widths) so one SPMD program serves all 8 cores.
"""

import numpy as np

import concourse.bacc as bacc
import concourse.bass as bass
import concourse.mybir as mybir
import concourse.tile as tile
from concourse import bass_utils

GX, GY = 480, 360
B = 2
EPS = 1e-5
N_CORES = 8
CELLS_PER_CORE = (B * GX * GY) // N_CORES  # 43200
CHUNK = 2048  # cells per chunk (fin granularity; 4 PSUM banks wide)
PAD = -1

F32 = mybir.dt.float32
BF16 = mybir.dt.bfloat16

Relu = mybir.ActivationFunctionType.Relu


# ---------------------------------------------------------------- host prep


def _build_plan_and_data(voxels, coors):
    """Route points to cores; build the equalized slot plan and per-core
    device inputs."""
    seg = (
        coors[:, 0].astype(np.int64) * (GX * GY)
        + coors[:, 1].astype(np.int64) * GY
        + coors[:, 2].astype(np.int64)
    )
    core_of = seg // CELLS_PER_CORE

    per_core = []
    for c in range(N_CORES):
        idx = np.nonzero(core_of == c)[0]
        seg_local = seg[idx] - c * CELLS_PER_CORE
        order = np.argsort(seg_local, kind="stable")
        seg_sorted = seg_local[order]
        cells, starts, counts = np.unique(
            seg_sorted, return_index=True, return_counts=True
        )
        o2 = np.argsort(-counts, kind="stable")
        cells, starts, counts = cells[o2], starts[o2], counts[o2]
        pts = idx[order]
        per_core.append((cells, starts, counts, pts))

    n_occ_max = max(len(pc[0]) for pc in per_core)
    G = -(-n_occ_max // CHUNK)
    if G % 2:
        G += 1  # fin works on chunk pairs
    kmax = int(max(pc[2].max() for pc in per_core))

    widths = np.zeros((kmax, G), np.int64)
    for cells, starts, counts, pts in per_core:
        n = len(cells)
        for g in range(G):
            lo, hi = g * CHUNK, min((g + 1) * CHUNK, n)
            if lo >= hi:
                continue
            cg = counts[lo:hi]
            for s in range(int(cg[0])):
                widths[s, g] = max(widths[s, g], int((cg > s).sum()))
    widths = np.minimum(-(-widths // 256) * 256, CHUNK)

    # items in s-major order; greedy-pack h1 blocks (w//2 cols) into
    # 2048-col groups so every item's h1 is contiguous in one tile
    items = []  # (s, g, w, grp, off)
    grp, off = 0, 0
    for s in range(kmax):
        for g in range(G):
            w = int(widths[s, g])
            if w == 0:
                continue
            h = w // 2
            if off + h > 1024:
                grp += 1
                off = 0
            items.append((s, g, w, grp, off))
            off += h
    G1 = grp + 1

    vox_all = np.zeros((N_CORES, 8, G1 * 1024), np.float32)
    rows_all = np.full((N_CORES, G * CHUNK), PAD, np.int64)

    for core, (cells, starts, counts, pts) in enumerate(per_core):
        n = len(cells)
        rows_all[core, :n] = cells
        for (s, g, w, gp, of) in items:
            lo = g * CHUNK
            ncell = max(0, min(w, n - lo))
            if ncell == 0:
                continue
            cnt = counts[lo : lo + ncell]
            so = np.minimum(s, cnt - 1)
            p_idx = pts[starts[lo : lo + ncell] + so]
            feats = np.zeros((w, 4), np.float32)
            feats[:ncell] = voxels[p_idx]
            h = w // 2
            packed = np.concatenate([feats[:h].T, feats[h:].T], axis=0)
            pc0 = gp * 1024 + of
            vox_all[core, :, pc0 : pc0 + h] = packed
    return (G, kmax, tuple(widths.flatten().tolist())), items, G1, \
        vox_all, rows_all


def _fold_weights(w1, g1, b1, m1, v1, w2, g2, b2, m2, v2,
                  w3, g3, b3, m3, v3, wc, bc):
    """Fold BN (eval) into the linear weights, build device layouts."""
    import ml_dtypes

    def sb(g, b, m, v):
        s = g / np.sqrt(v + EPS)
        return s.astype(np.float32), (b - m * s).astype(np.float32)

    s1, t1 = sb(g1, b1, m1, v1)
    s2, t2 = sb(g2, b2, m2, v2)
    s3, t3 = sb(g3, b3, m3, v3)
    w1f = (w1 * s1).astype(np.float32)  # [4,64]
    w2f = (w2 * s2).astype(np.float32)  # [64,128]
    w3f = (w3 * s3).astype(np.float32)  # [128,256]

    bf = ml_dtypes.bfloat16
    w1d8 = np.zeros((8, 128), np.float32)
    w1d8[0:4, 0:64] = w1f
    w1d8[4:8, 64:128] = w1f
    w2e = np.zeros((128, 128), np.float32)
    w2e[0:64] = w2f
    w2o = np.zeros((128, 128), np.float32)
    w2o[64:128] = w2f
    out = {
        "w1d8": w1d8.astype(bf),
        "w2e": w2e.astype(bf),
        "w2o": w2o.astype(bf),
        "w3a": np.ascontiguousarray(w3f[:, 0:128]).astype(bf),
        "w3b": np.ascontiguousarray(w3f[:, 128:256]).astype(bf),
        "wc0": np.ascontiguousarray(wc[0:128]).astype(np.float32).astype(bf),
        "wc1": np.ascontiguousarray(wc[128:256]).astype(np.float32).astype(bf),
        "t1d2": np.concatenate([t1, t1])[:, None].astype(np.float32),
        "t2": t2[:, None].astype(np.float32),
        "t3a": t3[0:128, None].astype(np.float32),
        "t3b": t3[128:256, None].astype(np.float32),
        "bc2": np.concatenate([bc, bc])[:, None].astype(np.float32),
    }
    return out


# ------------------------------------------------------------- bass program


def _sub_items(items):
    """Split each item into sub-items of <= 1024 h2 cols.

    Sub-item: (s, g, acc_off, wsub, segs) where segs is a list of
    (half, h1_lo, h1_len, dst_off) mapping h1 ranges (half 0 = even
    rows 0:64, 1 = odd rows 64:128) to the sub-item's h2 cols.
    """
    subs = []
    for (s, g, w, gp, of) in items:
        h = w // 2
        # concatenated h2 col space: [0:h) even, [h:w) odd
        u = 0
        while u * 1024 < w:
            lo, hi = u * 1024, min((u + 1) * 1024, w)
            segs = []
            for half, base in ((0, 0), (1, h)):
                a = max(lo, base)
                b = min(hi, base + h)
                if a < b:
                    segs.append((half, of + a - base, b - a, a - lo))
            subs.append((s, g, gp, u * 1024, hi - lo, segs))
            u += 1
    return subs


def build_program(G, items, G1):
    nc = bacc.Bacc("TRN2", target_bir_lowering=False, debug=False,
                   num_devices=N_CORES)

    vox = nc.dram_tensor("vox", [8, G1 * 1024], BF16,
                         kind="ExternalInput").ap()
    wdram = {}
    for name, shape, dt in [
        ("w1d8", [8, 128], BF16), ("w2e", [128, 128], BF16),
        ("w2o", [128, 128], BF16),
        ("w3a", [128, 128], BF16), ("w3b", [128, 128], BF16),
        ("wc0", [128, 64], BF16), ("wc1", [128, 64], BF16),
        ("t1d2", [128, 1], F32), ("t2", [128, 1], F32),
        ("t3a", [128, 1], F32), ("t3b", [128, 1], F32),
        ("bc2", [128, 1], F32),
    ]:
        wdram[name] = (nc.dram_tensor(name, shape, dt,
                                      kind="ExternalInput").ap(), shape, dt)
    comp = nc.dram_tensor("comp", [64, G * CHUNK], BF16,
                          kind="ExternalOutput").ap()

    from contextlib import ExitStack
    with tile.TileContext(nc) as tc, ExitStack() as ctx:
        cpool = ctx.enter_context(tc.tile_pool(name="const", bufs=1))

        _q = [nc.scalar, nc.gpsimd, nc.sync]
        wt = {}
        for i, name in enumerate(["w1d8", "t1d2", "w2e", "w2o", "t2",
                                  "w3a", "w3b", "t3a", "t3b",
                                  "wc0", "wc1", "bc2"]):
            ap, shape, dt = wdram[name]
            t = cpool.tile(shape, dt, tag=name)
            _q[i % 3].dma_start(out=t[:], in_=ap)
            wt[name] = t

        vox_sb = cpool.tile([8, G1 * 1024], BF16)
        for i in range(3):
            lo = (G1 * 1024 // 3) * i
            hi = (G1 * 1024 // 3) * (i + 1) if i < 2 else G1 * 1024
            _q[i].dma_start(out=vox_sb[:, lo:hi], in_=vox[:, lo:hi])

        accA = cpool.tile([128, G * CHUNK], BF16)
        accB = cpool.tile([128, G * CHUNK], BF16)
        w0 = {}
        for (s, g, w, gp, of) in items:
            w0[g] = max(w0.get(g, 0), w)
        for g in range(G):
            wg = w0.get(g, 0)
            if wg < CHUNK:
                nc.vector.memset(accA[:, g * CHUNK + wg : (g + 1) * CHUNK], 0.0)
                nc.vector.memset(accB[:, g * CHUNK + wg : (g + 1) * CHUNK], 0.0)

        h1p = ctx.enter_context(tc.tile_pool(name="h1p", bufs=6))
        h2p = ctx.enter_context(tc.tile_pool(name="h2p", bufs=6))
        tmp = ctx.enter_context(tc.tile_pool(name="tmp", bufs=4))
        scp = ctx.enter_context(tc.tile_pool(name="scp", bufs=3))
        psum = ctx.enter_context(tc.tile_pool(name="psum", bufs=4,
                                              space="PSUM"))

        debt = {"act": 0.0, "dve": 0.0}
        COST = {
            "act": lambda n: (n + 352.0) / 1.46 + 270.0,
            "dve": lambda n: (n + 120.0) / 1.13 + 110.0,
        }

        def br_auto(out_ap, in_ap, bias_ap):
            n = in_ap.shape[-1]
            eng = min(("act", "dve"), key=lambda e: debt[e] + COST[e](n))
            debt[eng] += COST[eng](n)
            if eng == "act":
                nc.scalar.activation(out_ap, in_ap, Relu, bias=bias_ap,
                                     scale=1.0)
            else:
                nc.vector.tensor_scalar(out_ap, in_ap, bias_ap, 0.0,
                                        op0=mybir.AluOpType.add,
                                        op1=mybir.AluOpType.max)

        h1t = {}

        def unit_p1(t):
            p = psum.tile([128, 1024], F32, tag="ps", space="PSUM",
                          name=f"p1_{t}")
            slab = vox_sb[:, t * 1024 : (t + 1) * 1024]
            for i in range(2):
                nc.tensor.matmul(p[:, i * 512 : (i + 1) * 512],
                                 wt["w1d8"][:],
                                 slab[:, i * 512 : (i + 1) * 512],
                                 start=True, stop=True)
            h1 = h1p.tile([128, 1024], BF16, tag="h1", name=f"h1_{t}")
            br_auto(h1[:], p[:], wt["t1d2"][:])
            h1t[t] = h1

        def seg512(c0, width):
            j = c0
            while j < c0 + width:
                j1 = min((j // 512 + 1) * 512, c0 + width)
                yield j, j1
                j = j1

        def unit_p2(sub):
            (s, g, gp, aoff, wsub, segs) = sub
            h1s = h1t[gp]
            p = psum.tile([128, 1024], F32, tag="ps", space="PSUM",
                          name=f"p2_{s}_{g}_{aoff}")[:, 0:wsub]
            for (half, lo, ln, doff) in segs:
                wst = wt["w2e"] if half == 0 else wt["w2o"]
                for j, j1 in seg512(doff, ln):
                    nc.tensor.matmul(p[:, j:j1], wst[:],
                                     h1s[:, lo + j - doff : lo + j1 - doff],
                                     start=True, stop=True)
            h2 = h2p.tile([128, 1024], BF16, tag="h2",
                          name=f"h2_{s}_{g}_{aoff}")[:, 0:wsub]
            br_auto(h2, p, wt["t2"][:])
            return h2

        def unit_p3(sub, h2_ap, half):
            (s, g, gp, aoff, wsub, segs) = sub
            wst = wt["w3a"] if half == 0 else wt["w3b"]
            bias = wt["t3a"] if half == 0 else wt["t3b"]
            base = g * CHUNK + aoff
            acc = (accA if half == 0 else accB)[:, base : base + wsub]
            p = psum.tile([128, 1024], F32, tag="ps", space="PSUM",
                          name=f"p3{half}_{s}_{g}_{aoff}")[:, 0:wsub]
            for j, j1 in seg512(0, wsub):
                nc.tensor.matmul(p[:, j:j1], wst[:], h2_ap[:, j:j1],
                                 start=True, stop=True)
            if s == 0:
                br_auto(acc, p, bias[:])
            else:
                tt = tmp.tile([128, 1024], BF16, tag="tt",
                              name=f"tt{half}_{s}_{g}_{aoff}")[:, 0:wsub]
                br_auto(tt, p, bias[:])
                debt["dve"] += (wsub / 2.0 + 151.0) / 0.96 + 110.0
                nc.vector.tensor_max(acc, tt, acc)

        def unit_fin(g):
            for j in (0, 1024):
                p = psum.tile([128, 1024], F32, tag="ps", space="PSUM",
                              name=f"pc_{g}_{j}")
                for j2 in (0, 512):
                    for q in range(2):
                        cols = (g + q) * CHUNK + j + j2
                        dst = p[64 * q : 64 * q + 64, j2 : j2 + 512]
                        nc.tensor.matmul(dst, wt["wc0"][:],
                                         accA[:, cols : cols + 512],
                                         start=True, stop=False)
                        nc.tensor.matmul(dst, wt["wc1"][:],
                                         accB[:, cols : cols + 512],
                                         start=False, stop=True)
                sc = scp.tile([128, 1024], BF16, tag="sc", name=f"sc_{g}_{j}")
                br_auto(sc[:], p[:], wt["bc2"][:])
                for q in range(2):
                    o = (g + q) * CHUNK + j
                    nc.gpsimd.dma_start(out=comp[:, o : o + 1024],
                                        in_=sc[64 * q : 64 * q + 64, :])

        # ---- software-pipelined emission over sub-items ----
        subs = _sub_items(items)
        last_sub = {}
        for i, sub in enumerate(subs):
            last_sub[sub[1]] = i  # last sub index per chunk
        done = [False] * G
        finned = [False] * G

        def try_fins(g):
            done[g] = True
            gp = g - (g % 2)
            if done[gp] and done[gp + 1] and not finned[gp]:
                finned[gp] = True
                unit_fin(gp)

        have = {sub[1] for sub in subs}
        for g in range(G):
            if g not in have:
                done[g] = True
        for g in range(0, G, 2):
            if done[g] and done[g + 1]:
                finned[g] = True

        p2q = list(enumerate(subs))
        p3q = []

        def pump(grp_done_upto):
            while p3q:
                i, sub, h2a = p3q.pop(0)
                unit_p3(sub, h2a, 0)
                unit_p3(sub, h2a, 1)
                if i == last_sub[sub[1]]:
                    try_fins(sub[1])
            while p2q and p2q[0][1][2] < grp_done_upto:
                i, sub = p2q.pop(0)
                p3q.append((i, sub, unit_p2(sub)))

        for t in range(G1):
            unit_p1(t)
            pump(t)
        pump(G1)
        pump(G1)

    nc.compile()
    return nc


# ------------------------------------------------------------------ driver

_CACHE = {}


def kernel(voxels, coors, batch_size, w1, g1, b1, m1, v1,
           w2, g2, b2, m2, v2, w3, g3, b3, m3, v3, wc, bc,
           _trace=False):
    voxels = np.asarray(voxels, np.float32)
    coors = np.asarray(coors, np.int32)
    plan_key, items, G1, vox_all, rows_all = _build_plan_and_data(
        voxels, coors)
    G = plan_key[0]

    if plan_key not in _CACHE:
        _CACHE[plan_key] = build_program(G, items, G1)
    nc = _CACHE[plan_key]

    folded = _fold_weights(
        np.asarray(w1, np.float32), np.asarray(g1, np.float32),
        np.asarray(b1, np.float32), np.asarray(m1, np.float32),
        np.asarray(v1, np.float32),
        np.asarray(w2, np.float32), np.asarray(g2, np.float32),
        np.asarray(b2, np.float32), np.asarray(m2, np.float32),
        np.asarray(v2, np.float32),
        np.asarray(w3, np.float32), np.asarray(g3, np.float32),
        np.asarray(b3, np.float32), np.asarray(m3, np.float32),
        np.asarray(v3, np.float32),
        np.asarray(wc, np.float32), np.asarray(bc, np.float32))

    import ml_dtypes
    in_maps = [
        {"vox": vox_all[c].astype(ml_dtypes.bfloat16), **folded}
        for c in range(N_CORES)
    ]
    res = bass_utils.run_bass_kernel_spmd(
        nc, in_maps, core_ids=list(range(N_CORES)), trace=_trace)

    out = np.zeros((B, 64, GX * GY), np.float32)
    for c in range(N_CORES):
        cm = np.asarray(res.results[c]["comp"]).astype(np.float32)
        rows = rows_all[c]
        real = rows != PAD
        gcell = rows[real] + c * CELLS_PER_CORE
        b_core = c // (N_CORES // B)
        xy = gcell - b_core * (GX * GY)
        out[b_core][:, xy] = cm[:, real]
    out = out.reshape(B, 64, GX, GY)
    if _trace:
        return out, res
    return out


# revision 22
# speedup vs baseline: 1.4825x; 1.0218x over previous
"""BevFeatureEncoder on 8 Trainium2 NeuronCores.

Strategy (data-parallel over BEV grid slabs):
  - The 2*480*360 BEV cells are split into 8 contiguous ranges of 43200
    cells; points are routed on host to the core owning their cell, so
    the segment_max reduction is fully local to each core.
  - Per core, occupied cells are sorted by point count DESC and grouped
    into chunks of 2048 cells. Slot s of a chunk covers only the prefix
    of cells with count > s (widths shrink with s, rounded to 256), so
    there is no power-of-2 slot padding. Items run in s-major order so
    the per-chunk max-accumulate chains never serialize back-to-back.
  - BN scale/shift is folded into the weights ON HOST (numpy): the
    device sees pre-folded bf16 stationaries + per-partition biases and
    has no weight-prep preamble.
  - Device dataflow: 2048-wide units, each 4 matmuls into a 4-bank PSUM
    tile + ONE wide drain (relu+bias) on ACT or DVE, debt-balanced.
    Layer 1 packs two cells per column (block-diag [8,128]); layer 2
    unpacks via even/odd zero-padded stationaries; the compression runs
    col-tiled (out partitions 0:64 / 64:128 for a chunk pair). Max
    accumulation (slots s>0) is split: ACT/DVE relu-drain to a temp
    tile, then a 2x-rate bf16 SBUF tensor_tensor max into the
    accumulator planes.
  - Output is compacted [64, G*2048] bf16; host places real columns
    into the zeroed [B, C, GX, GY] grid. Chunk structure is equalized
    across cores (max widths) so one SPMD program serves all 8 cores.
"""

import numpy as np

import concourse.bacc as bacc
import concourse.bass as bass
import concourse.mybir as mybir
import concourse.tile as tile
from concourse import bass_utils

GX, GY = 480, 360
B = 2
EPS = 1e-5
N_CORES = 8
CELLS_PER_CORE = (B * GX * GY) // N_CORES  # 43200
CHUNK = 2048  # cells per chunk (fin granularity; 4 PSUM banks wide)
PAD = -1

F32 = mybir.dt.float32
BF16 = mybir.dt.bfloat16

Relu = mybir.ActivationFunctionType.Relu


# ---------------------------------------------------------------- host prep


def _build_plan_and_data(voxels, coors):
    """Route points to cores; build the equalized slot plan and per-core
    device inputs."""
    seg = (
        coors[:, 0].astype(np.int64) * (GX * GY)
        + coors[:, 1].astype(np.int64) * GY
        + coors[:, 2].astype(np.int64)
    )
    core_of = seg // CELLS_PER_CORE

    per_core = []
    for c in range(N_CORES):
        idx = np.nonzero(core_of == c)[0]
        seg_local = seg[idx] - c * CELLS_PER_CORE
        order = np.argsort(seg_local, kind="stable")
        seg_sorted = seg_local[order]
        cells, starts, counts = np.unique(
            seg_sorted, return_index=True, return_counts=True
        )
        o2 = np.argsort(-counts, kind="stable")
        cells, starts, counts = cells[o2], starts[o2], counts[o2]
        pts = idx[order]
        per_core.append((cells, starts, counts, pts))

    n_occ_max = max(len(pc[0]) for pc in per_core)
    G = -(-n_occ_max // CHUNK)
    if G % 2:
        G += 1  # fin works on chunk pairs
    kmax = int(max(pc[2].max() for pc in per_core))

    widths = np.zeros((kmax, G), np.int64)
    for cells, starts, counts, pts in per_core:
        n = len(cells)
        for g in range(G):
            lo, hi = g * CHUNK, min((g + 1) * CHUNK, n)
            if lo >= hi:
                continue
            cg = counts[lo:hi]
            for s in range(int(cg[0])):
                widths[s, g] = max(widths[s, g], int((cg > s).sum()))
    widths = np.minimum(-(-widths // 256) * 256, CHUNK)

    # items in s-major order; greedy-pack h1 blocks (w//2 cols) into
    # 2048-col groups so every item's h1 is contiguous in one tile
    items = []  # (s, g, w, grp, off)
    grp, off = 0, 0
    for s in range(kmax):
        for g in range(G):
            w = int(widths[s, g])
            if w == 0:
                continue
            h = w // 2
            if off + h > 1024:
                grp += 1
                off = 0
            items.append((s, g, w, grp, off))
            off += h
    G1 = grp + 1

    vox_all = np.zeros((N_CORES, 8, G1 * 1024), np.float32)
    rows_all = np.full((N_CORES, G * CHUNK), PAD, np.int64)

    for core, (cells, starts, counts, pts) in enumerate(per_core):
        n = len(cells)
        rows_all[core, :n] = cells
        for (s, g, w, gp, of) in items:
            lo = g * CHUNK
            ncell = max(0, min(w, n - lo))
            if ncell == 0:
                continue
            cnt = counts[lo : lo + ncell]
            so = np.minimum(s, cnt - 1)
            p_idx = pts[starts[lo : lo + ncell] + so]
            feats = np.zeros((w, 4), np.float32)
            feats[:ncell] = voxels[p_idx]
            h = w // 2
            packed = np.concatenate([feats[:h].T, feats[h:].T], axis=0)
            pc0 = gp * 1024 + of
            vox_all[core, :, pc0 : pc0 + h] = packed
    return (G, kmax, tuple(widths.flatten().tolist())), items, G1, \
        vox_all, rows_all


def _fold_weights(w1, g1, b1, m1, v1, w2, g2, b2, m2, v2,
                  w3, g3, b3, m3, v3, wc, bc):
    """Fold BN (eval) into the linear weights, build device layouts."""
    import ml_dtypes

    def sb(g, b, m, v):
        s = g / np.sqrt(v + EPS)
        return s.astype(np.float32), (b - m * s).astype(np.float32)

    s1, t1 = sb(g1, b1, m1, v1)
    s2, t2 = sb(g2, b2, m2, v2)
    s3, t3 = sb(g3, b3, m3, v3)
    w1f = (w1 * s1).astype(np.float32)  # [4,64]
    w2f = (w2 * s2).astype(np.float32)  # [64,128]
    w3f = (w3 * s3).astype(np.float32)  # [128,256]

    bf = ml_dtypes.bfloat16
    w1d8 = np.zeros((8, 128), np.float32)
    w1d8[0:4, 0:64] = w1f
    w1d8[4:8, 64:128] = w1f
    w2e = np.zeros((128, 128), np.float32)
    w2e[0:64] = w2f
    w2o = np.zeros((128, 128), np.float32)
    w2o[64:128] = w2f
    out = {
        "w1d8": w1d8.astype(bf),
        "w2e": w2e.astype(bf),
        "w2o": w2o.astype(bf),
        "w3a": np.ascontiguousarray(w3f[:, 0:128]).astype(bf),
        "w3b": np.ascontiguousarray(w3f[:, 128:256]).astype(bf),
        "wc0": np.ascontiguousarray(wc[0:128]).astype(np.float32).astype(bf),
        "wc1": np.ascontiguousarray(wc[128:256]).astype(np.float32).astype(bf),
        "t1d2": np.concatenate([t1, t1])[:, None].astype(np.float32),
        "t2": t2[:, None].astype(np.float32),
        "t3a": t3[0:128, None].astype(np.float32),
        "t3b": t3[128:256, None].astype(np.float32),
        "bc2": np.concatenate([bc, bc])[:, None].astype(np.float32),
    }
    return out


# ------------------------------------------------------------- bass program


def _sub_items(items):
    """Split each item into sub-items of <= 1024 h2 cols.

    Sub-item: (s, g, acc_off, wsub, segs) where segs is a list of
    (half, h1_lo, h1_len, dst_off) mapping h1 ranges (half 0 = even
    rows 0:64, 1 = odd rows 64:128) to the sub-item's h2 cols.
    """
    subs = []
    for (s, g, w, gp, of) in items:
        h = w // 2
        # concatenated h2 col space: [0:h) even, [h:w) odd
        u = 0
        while u * 1024 < w:
            lo, hi = u * 1024, min((u + 1) * 1024, w)
            segs = []
            for half, base in ((0, 0), (1, h)):
                a = max(lo, base)
                b = min(hi, base + h)
                if a < b:
                    segs.append((half, of + a - base, b - a, a - lo))
            subs.append((s, g, gp, u * 1024, hi - lo, segs))
            u += 1
    return subs


def build_program(G, items, G1):
    nc = bacc.Bacc("TRN2", target_bir_lowering=False, debug=False,
                   num_devices=N_CORES)

    vox = nc.dram_tensor("vox", [8, G1 * 1024], BF16,
                         kind="ExternalInput").ap()
    wdram = {}
    for name, shape, dt in [
        ("w1d8", [8, 128], BF16), ("w2e", [128, 128], BF16),
        ("w2o", [128, 128], BF16),
        ("w3a", [128, 128], BF16), ("w3b", [128, 128], BF16),
        ("wc0", [128, 64], BF16), ("wc1", [128, 64], BF16),
        ("t1d2", [128, 1], F32), ("t2", [128, 1], F32),
        ("t3a", [128, 1], F32), ("t3b", [128, 1], F32),
        ("bc2", [128, 1], F32),
    ]:
        wdram[name] = (nc.dram_tensor(name, shape, dt,
                                      kind="ExternalInput").ap(), shape, dt)
    comp = nc.dram_tensor("comp", [64, G * CHUNK], BF16,
                          kind="ExternalOutput").ap()

    from contextlib import ExitStack
    with tile.TileContext(nc) as tc, ExitStack() as ctx:
        cpool = ctx.enter_context(tc.tile_pool(name="const", bufs=1))

        _q = [nc.scalar, nc.gpsimd, nc.sync]
        wt = {}
        for i, name in enumerate(["w1d8", "t1d2", "w2e", "w2o", "t2",
                                  "w3a", "w3b", "t3a", "t3b",
                                  "wc0", "wc1", "bc2"]):
            ap, shape, dt = wdram[name]
            t = cpool.tile(shape, dt, tag=name)
            _q[i % 3].dma_start(out=t[:], in_=ap)
            wt[name] = t

        vox_sb = cpool.tile([8, G1 * 1024], BF16)
        # many small DMAs in consumption order so the first p1 units
        # start immediately; spread across the three DMA-capable queues
        NPC = max(1, G1 // 6)
        bounds = list(range(0, G1, NPC)) + [G1]
        for i in range(len(bounds) - 1):
            lo, hi = bounds[i] * 1024, bounds[i + 1] * 1024
            _q[i % 3].dma_start(out=vox_sb[:, lo:hi], in_=vox[:, lo:hi])

        accA = cpool.tile([128, G * CHUNK], BF16)
        accB = cpool.tile([128, G * CHUNK], BF16)
        w0 = {}
        for (s, g, w, gp, of) in items:
            w0[g] = max(w0.get(g, 0), w)
        for g in range(G):
            wg = w0.get(g, 0)
            if wg < CHUNK:
                nc.vector.memset(accA[:, g * CHUNK + wg : (g + 1) * CHUNK], 0.0)
                nc.vector.memset(accB[:, g * CHUNK + wg : (g + 1) * CHUNK], 0.0)

        h1p = ctx.enter_context(tc.tile_pool(name="h1p", bufs=6))
        h2p = ctx.enter_context(tc.tile_pool(name="h2p", bufs=6))
        tmp = ctx.enter_context(tc.tile_pool(name="tmp", bufs=4))
        scp = ctx.enter_context(tc.tile_pool(name="scp", bufs=3))
        psum = ctx.enter_context(tc.tile_pool(name="psum", bufs=4,
                                              space="PSUM"))

        debt = {"act": 0.0, "dve": 0.0}
        COST = {
            "act": lambda n: (n + 352.0) / 1.33 + 184.0,
            "dve": lambda n: (n + 120.0) / 0.928 + 126.0,
        }

        def br_auto(out_ap, in_ap, bias_ap):
            n = in_ap.shape[-1]
            eng = min(("act", "dve"), key=lambda e: debt[e] + COST[e](n))
            debt[eng] += COST[eng](n)
            if eng == "act":
                nc.scalar.activation(out_ap, in_ap, Relu, bias=bias_ap,
                                     scale=1.0)
            else:
                nc.vector.tensor_scalar(out_ap, in_ap, bias_ap, 0.0,
                                        op0=mybir.AluOpType.add,
                                        op1=mybir.AluOpType.max)

        h1t = {}

        def unit_p1(t):
            p = psum.tile([128, 1024], F32, tag="ps", space="PSUM",
                          name=f"p1_{t}")
            slab = vox_sb[:, t * 1024 : (t + 1) * 1024]
            for i in range(2):
                nc.tensor.matmul(p[:, i * 512 : (i + 1) * 512],
                                 wt["w1d8"][:],
                                 slab[:, i * 512 : (i + 1) * 512],
                                 start=True, stop=True)
            h1 = h1p.tile([128, 1024], BF16, tag="h1", name=f"h1_{t}")
            br_auto(h1[:], p[:], wt["t1d2"][:])
            h1t[t] = h1

        def seg512(c0, width):
            j = c0
            while j < c0 + width:
                j1 = min((j // 512 + 1) * 512, c0 + width)
                yield j, j1
                j = j1

        def unit_p2(sub):
            (s, g, gp, aoff, wsub, segs) = sub
            h1s = h1t[gp]
            p = psum.tile([128, 1024], F32, tag="ps", space="PSUM",
                          name=f"p2_{s}_{g}_{aoff}")[:, 0:wsub]
            for (half, lo, ln, doff) in segs:
                wst = wt["w2e"] if half == 0 else wt["w2o"]
                for j, j1 in seg512(doff, ln):
                    nc.tensor.matmul(p[:, j:j1], wst[:],
                                     h1s[:, lo + j - doff : lo + j1 - doff],
                                     start=True, stop=True)
            h2 = h2p.tile([128, 1024], BF16, tag="h2",
                          name=f"h2_{s}_{g}_{aoff}")[:, 0:wsub]
            br_auto(h2, p, wt["t2"][:])
            return h2

        def unit_p3(sub, h2_ap, half):
            (s, g, gp, aoff, wsub, segs) = sub
            wst = wt["w3a"] if half == 0 else wt["w3b"]
            bias = wt["t3a"] if half == 0 else wt["t3b"]
            base = g * CHUNK + aoff
            acc = (accA if half == 0 else accB)[:, base : base + wsub]
            p = psum.tile([128, 1024], F32, tag="ps", space="PSUM",
                          name=f"p3{half}_{s}_{g}_{aoff}")[:, 0:wsub]
            for j, j1 in seg512(0, wsub):
                nc.tensor.matmul(p[:, j:j1], wst[:], h2_ap[:, j:j1],
                                 start=True, stop=True)
            if s == 0:
                br_auto(acc, p, bias[:])
            else:
                tt = tmp.tile([128, 1024], BF16, tag="tt",
                              name=f"tt{half}_{s}_{g}_{aoff}")[:, 0:wsub]
                br_auto(tt, p, bias[:])
                debt["dve"] += (wsub / 2.0 + 151.0) / 0.96 + 126.0
                nc.vector.tensor_max(acc, tt, acc)

        def unit_fin(g):
            for j in (0, 1024):
                p = psum.tile([128, 1024], F32, tag="ps", space="PSUM",
                              name=f"pc_{g}_{j}")
                for j2 in (0, 512):
                    for q in range(2):
                        cols = (g + q) * CHUNK + j + j2
                        dst = p[64 * q : 64 * q + 64, j2 : j2 + 512]
                        nc.tensor.matmul(dst, wt["wc0"][:],
                                         accA[:, cols : cols + 512],
                                         start=True, stop=False)
                        nc.tensor.matmul(dst, wt["wc1"][:],
                                         accB[:, cols : cols + 512],
                                         start=False, stop=True)
                sc = scp.tile([128, 1024], BF16, tag="sc", name=f"sc_{g}_{j}")
                br_auto(sc[:], p[:], wt["bc2"][:])
                for q in range(2):
                    o = (g + q) * CHUNK + j
                    nc.gpsimd.dma_start(out=comp[:, o : o + 1024],
                                        in_=sc[64 * q : 64 * q + 64, :])

        # ---- software-pipelined emission over sub-items ----
        subs = _sub_items(items)
        last_sub = {}
        for i, sub in enumerate(subs):
            last_sub[sub[1]] = i  # last sub index per chunk
        done = [False] * G
        finned = [False] * G

        def try_fins(g):
            done[g] = True
            gp = g - (g % 2)
            if done[gp] and done[gp + 1] and not finned[gp]:
                finned[gp] = True
                unit_fin(gp)

        have = {sub[1] for sub in subs}
        for g in range(G):
            if g not in have:
                done[g] = True
        for g in range(0, G, 2):
            if done[g] and done[g + 1]:
                finned[g] = True

        p2q = list(enumerate(subs))
        p3q = []

        def pump(grp_done_upto):
            while p3q:
                i, sub, h2a = p3q.pop(0)
                unit_p3(sub, h2a, 0)
                unit_p3(sub, h2a, 1)
                if i == last_sub[sub[1]]:
                    try_fins(sub[1])
            while p2q and p2q[0][1][2] < grp_done_upto:
                i, sub = p2q.pop(0)
                p3q.append((i, sub, unit_p2(sub)))

        for t in range(G1):
            unit_p1(t)
            pump(t)
        pump(G1)
        pump(G1)

    nc.compile()
    return nc


# ------------------------------------------------------------------ driver

_CACHE = {}


def kernel(voxels, coors, batch_size, w1, g1, b1, m1, v1,
           w2, g2, b2, m2, v2, w3, g3, b3, m3, v3, wc, bc,
           _trace=False):
    voxels = np.asarray(voxels, np.float32)
    coors = np.asarray(coors, np.int32)
    plan_key, items, G1, vox_all, rows_all = _build_plan_and_data(
        voxels, coors)
    G = plan_key[0]

    if plan_key not in _CACHE:
        _CACHE[plan_key] = build_program(G, items, G1)
    nc = _CACHE[plan_key]

    folded = _fold_weights(
        np.asarray(w1, np.float32), np.asarray(g1, np.float32),
        np.asarray(b1, np.float32), np.asarray(m1, np.float32),
        np.asarray(v1, np.float32),
        np.asarray(w2, np.float32), np.asarray(g2, np.float32),
        np.asarray(b2, np.float32), np.asarray(m2, np.float32),
        np.asarray(v2, np.float32),
        np.asarray(w3, np.float32), np.asarray(g3, np.float32),
        np.asarray(b3, np.float32), np.asarray(m3, np.float32),
        np.asarray(v3, np.float32),
        np.asarray(wc, np.float32), np.asarray(bc, np.float32))

    import ml_dtypes
    in_maps = [
        {"vox": vox_all[c].astype(ml_dtypes.bfloat16), **folded}
        for c in range(N_CORES)
    ]
    res = bass_utils.run_bass_kernel_spmd(
        nc, in_maps, core_ids=list(range(N_CORES)), trace=_trace)

    out = np.zeros((B, 64, GX * GY), np.float32)
    for c in range(N_CORES):
        cm = np.asarray(res.results[c]["comp"]).astype(np.float32)
        rows = rows_all[c]
        real = rows != PAD
        gcell = rows[real] + c * CELLS_PER_CORE
        b_core = c // (N_CORES // B)
        xy = gcell - b_core * (GX * GY)
        out[b_core][:, xy] = cm[:, real]
    out = out.reshape(B, 64, GX, GY)
    if _trace:
        return out, res
    return out


# revision 25
# speedup vs baseline: 1.5118x; 1.0197x over previous
"""BevFeatureEncoder on 8 Trainium2 NeuronCores.

Strategy (data-parallel over BEV grid slabs):
  - The 2*480*360 BEV cells are split into 8 contiguous ranges of 43200
    cells; points are routed on host to the core owning their cell, so
    the segment_max reduction is fully local to each core.
  - Per core, occupied cells are sorted by point count DESC and grouped
    into chunks of 2048 cells. Slot s of a chunk covers only the prefix
    of cells with count > s (widths shrink with s, rounded to 256), so
    there is no power-of-2 slot padding. Items run in s-major order so
    the per-chunk max-accumulate chains never serialize back-to-back.
  - BN scale/shift is folded into the weights ON HOST (numpy): the
    device sees pre-folded bf16 stationaries + per-partition biases and
    has no weight-prep preamble.
  - Device dataflow: 2048-wide units, each 4 matmuls into a 4-bank PSUM
    tile + ONE wide drain (relu+bias) on ACT or DVE, debt-balanced.
    Layer 1 packs two cells per column (block-diag [8,128]); layer 2
    unpacks via even/odd zero-padded stationaries; the compression runs
    col-tiled (out partitions 0:64 / 64:128 for a chunk pair). Max
    accumulation (slots s>0) is split: ACT/DVE relu-drain to a temp
    tile, then a 2x-rate bf16 SBUF tensor_tensor max into the
    accumulator planes.
  - Output is compacted [64, G*2048] bf16; host places real columns
    into the zeroed [B, C, GX, GY] grid. Chunk structure is equalized
    across cores (max widths) so one SPMD program serves all 8 cores.
"""

import numpy as np

import concourse.bacc as bacc
import concourse.bass as bass
import concourse.mybir as mybir
import concourse.tile as tile
from concourse import bass_utils

GX, GY = 480, 360
B = 2
EPS = 1e-5
N_CORES = 8
CELLS_PER_CORE = (B * GX * GY) // N_CORES  # 43200
CHUNK = 2048  # cells per chunk (fin granularity; 4 PSUM banks wide)
PAD = -1

F32 = mybir.dt.float32
BF16 = mybir.dt.bfloat16

Relu = mybir.ActivationFunctionType.Relu


# ---------------------------------------------------------------- host prep


def _build_plan_and_data(voxels, coors):
    """Route points to cores; build the equalized slot plan and per-core
    device inputs."""
    seg = (
        coors[:, 0].astype(np.int64) * (GX * GY)
        + coors[:, 1].astype(np.int64) * GY
        + coors[:, 2].astype(np.int64)
    )
    core_of = seg // CELLS_PER_CORE

    per_core = []
    for c in range(N_CORES):
        idx = np.nonzero(core_of == c)[0]
        seg_local = seg[idx] - c * CELLS_PER_CORE
        order = np.argsort(seg_local, kind="stable")
        seg_sorted = seg_local[order]
        cells, starts, counts = np.unique(
            seg_sorted, return_index=True, return_counts=True
        )
        o2 = np.argsort(-counts, kind="stable")
        cells, starts, counts = cells[o2], starts[o2], counts[o2]
        pts = idx[order]
        per_core.append((cells, starts, counts, pts))

    n_occ_max = max(len(pc[0]) for pc in per_core)
    G = -(-n_occ_max // CHUNK)
    if G % 2:
        G += 1  # fin works on chunk pairs
    kmax = int(max(pc[2].max() for pc in per_core))

    widths = np.zeros((kmax, G), np.int64)
    for cells, starts, counts, pts in per_core:
        n = len(cells)
        for g in range(G):
            lo, hi = g * CHUNK, min((g + 1) * CHUNK, n)
            if lo >= hi:
                continue
            cg = counts[lo:hi]
            for s in range(int(cg[0])):
                widths[s, g] = max(widths[s, g], int((cg > s).sum()))
    widths = np.minimum(-(-widths // 256) * 256, CHUNK)

    # items in s-major order; greedy-pack h1 blocks (w//2 cols) into
    # 2048-col groups so every item's h1 is contiguous in one tile
    items = []  # (s, g, w, grp, off)
    grp, off = 0, 0
    for s in range(kmax):
        for g in range(G):
            w = int(widths[s, g])
            if w == 0:
                continue
            h = w // 2
            if off + h > 1024:
                grp += 1
                off = 0
            items.append((s, g, w, grp, off))
            off += h
    G1 = grp + 1

    vox_all = np.zeros((N_CORES, 8, G1 * 1024), np.float32)
    rows_all = np.full((N_CORES, G * CHUNK), PAD, np.int64)

    for core, (cells, starts, counts, pts) in enumerate(per_core):
        n = len(cells)
        rows_all[core, :n] = cells
        for (s, g, w, gp, of) in items:
            lo = g * CHUNK
            ncell = max(0, min(w, n - lo))
            if ncell == 0:
                continue
            cnt = counts[lo : lo + ncell]
            so = np.minimum(s, cnt - 1)
            p_idx = pts[starts[lo : lo + ncell] + so]
            feats = np.zeros((w, 4), np.float32)
            feats[:ncell] = voxels[p_idx]
            h = w // 2
            packed = np.concatenate([feats[:h].T, feats[h:].T], axis=0)
            pc0 = gp * 1024 + of
            vox_all[core, :, pc0 : pc0 + h] = packed
    return (G, kmax, tuple(widths.flatten().tolist())), items, G1, \
        vox_all, rows_all


def _fold_weights(w1, g1, b1, m1, v1, w2, g2, b2, m2, v2,
                  w3, g3, b3, m3, v3, wc, bc):
    """Fold BN (eval) into the linear weights, build device layouts."""
    import ml_dtypes

    def sb(g, b, m, v):
        s = g / np.sqrt(v + EPS)
        return s.astype(np.float32), (b - m * s).astype(np.float32)

    s1, t1 = sb(g1, b1, m1, v1)
    s2, t2 = sb(g2, b2, m2, v2)
    s3, t3 = sb(g3, b3, m3, v3)
    w1f = (w1 * s1).astype(np.float32)  # [4,64]
    w2f = (w2 * s2).astype(np.float32)  # [64,128]
    w3f = (w3 * s3).astype(np.float32)  # [128,256]

    bf = ml_dtypes.bfloat16
    w1d8 = np.zeros((8, 128), np.float32)
    w1d8[0:4, 0:64] = w1f
    w1d8[4:8, 64:128] = w1f
    w2e = np.zeros((128, 128), np.float32)
    w2e[0:64] = w2f
    w2o = np.zeros((128, 128), np.float32)
    w2o[64:128] = w2f
    out = {
        "w1d8": w1d8.astype(bf),
        "w2e": w2e.astype(bf),
        "w2o": w2o.astype(bf),
        "w3a": np.ascontiguousarray(w3f[:, 0:128]).astype(bf),
        "w3b": np.ascontiguousarray(w3f[:, 128:256]).astype(bf),
        "wc0": np.ascontiguousarray(wc[0:128]).astype(np.float32).astype(bf),
        "wc1": np.ascontiguousarray(wc[128:256]).astype(np.float32).astype(bf),
        "t1d2": np.concatenate([t1, t1])[:, None].astype(np.float32),
        "t2": t2[:, None].astype(np.float32),
        "t3a": t3[0:128, None].astype(np.float32),
        "t3b": t3[128:256, None].astype(np.float32),
        "bc2": np.concatenate([bc, bc])[:, None].astype(np.float32),
    }
    return out


# ------------------------------------------------------------- bass program


def _sub_items(items):
    """Split each item into sub-items of <= 1024 h2 cols.

    Sub-item: (s, g, acc_off, wsub, segs) where segs is a list of
    (half, h1_lo, h1_len, dst_off) mapping h1 ranges (half 0 = even
    rows 0:64, 1 = odd rows 64:128) to the sub-item's h2 cols.
    """
    subs = []
    for (s, g, w, gp, of) in items:
        h = w // 2
        # concatenated h2 col space: [0:h) even, [h:w) odd
        u = 0
        while u * 1024 < w:
            lo, hi = u * 1024, min((u + 1) * 1024, w)
            segs = []
            for half, base in ((0, 0), (1, h)):
                a = max(lo, base)
                b = min(hi, base + h)
                if a < b:
                    segs.append((half, of + a - base, b - a, a - lo))
            subs.append((s, g, gp, u * 1024, hi - lo, segs))
            u += 1
    return subs


def build_program(G, items, G1):
    nc = bacc.Bacc("TRN2", target_bir_lowering=False, debug=False,
                   num_devices=N_CORES)

    vox = nc.dram_tensor("vox", [8, G1 * 1024], BF16,
                         kind="ExternalInput").ap()
    wdram = {}
    for name, shape, dt in [
        ("w1d8", [8, 128], BF16), ("w2e", [128, 128], BF16),
        ("w2o", [128, 128], BF16),
        ("w3a", [128, 128], BF16), ("w3b", [128, 128], BF16),
        ("wc0", [128, 64], BF16), ("wc1", [128, 64], BF16),
        ("t1d2", [128, 1], F32), ("t2", [128, 1], F32),
        ("t3a", [128, 1], F32), ("t3b", [128, 1], F32),
        ("bc2", [128, 1], F32),
    ]:
        wdram[name] = (nc.dram_tensor(name, shape, dt,
                                      kind="ExternalInput").ap(), shape, dt)
    comp = nc.dram_tensor("comp", [64, G * CHUNK], BF16,
                          kind="ExternalOutput").ap()

    from contextlib import ExitStack
    with tile.TileContext(nc) as tc, ExitStack() as ctx:
        cpool = ctx.enter_context(tc.tile_pool(name="const", bufs=1))

        _q = [nc.scalar, nc.gpsimd, nc.sync]
        wt = {}
        for i, name in enumerate(["w1d8", "t1d2", "w2e", "w2o", "t2",
                                  "w3a", "w3b", "t3a", "t3b",
                                  "wc0", "wc1", "bc2"]):
            ap, shape, dt = wdram[name]
            t = cpool.tile(shape, dt, tag=name)
            _q[i % 3].dma_start(out=t[:], in_=ap)
            wt[name] = t

        vox_sb = cpool.tile([8, G1 * 1024], BF16)
        # many small DMAs in consumption order, spread across queues
        PIECE = 2048
        for i, c0 in enumerate(range(0, G1 * 1024, PIECE)):
            c1 = min(c0 + PIECE, G1 * 1024)
            _q[i % 3].dma_start(out=vox_sb[:, c0:c1], in_=vox[:, c0:c1])

        accA = cpool.tile([128, G * CHUNK], BF16)
        accB = cpool.tile([128, G * CHUNK], BF16)
        w0 = {}
        for (s, g, w, gp, of) in items:
            w0[g] = max(w0.get(g, 0), w)
        for g in range(G):
            wg = w0.get(g, 0)
            if wg < CHUNK:
                nc.vector.memset(accA[:, g * CHUNK + wg : (g + 1) * CHUNK], 0.0)
                nc.vector.memset(accB[:, g * CHUNK + wg : (g + 1) * CHUNK], 0.0)

        h1p = ctx.enter_context(tc.tile_pool(name="h1p", bufs=6))
        h2p = ctx.enter_context(tc.tile_pool(name="h2p", bufs=6))
        tmp = ctx.enter_context(tc.tile_pool(name="tmp", bufs=4))
        scp = ctx.enter_context(tc.tile_pool(name="scp", bufs=3))
        psum = ctx.enter_context(tc.tile_pool(name="psum", bufs=4,
                                              space="PSUM"))

        debt = {"act": 0.0, "dve": 0.0}
        COST = {
            "act": lambda n: (n + 352.0) / 1.33 + 250.0,
            "dve": lambda n: (n + 120.0) / 0.94 + 160.0,
        }

        def br_auto(out_ap, in_ap, bias_ap):
            n = in_ap.shape[-1]
            eng = min(("act", "dve"), key=lambda e: debt[e] + COST[e](n))
            debt[eng] += COST[eng](n)
            if eng == "act":
                nc.scalar.activation(out_ap, in_ap, Relu, bias=bias_ap,
                                     scale=1.0)
            else:
                nc.vector.tensor_scalar(out_ap, in_ap, bias_ap, 0.0,
                                        op0=mybir.AluOpType.add,
                                        op1=mybir.AluOpType.max)

        h1t = {}

        def unit_p1(t):
            p = psum.tile([128, 1024], F32, tag="ps", space="PSUM",
                          name=f"p1_{t}")
            slab = vox_sb[:, t * 1024 : (t + 1) * 1024]
            for i in range(2):
                nc.tensor.matmul(p[:, i * 512 : (i + 1) * 512],
                                 wt["w1d8"][:],
                                 slab[:, i * 512 : (i + 1) * 512],
                                 start=True, stop=True)
            h1 = h1p.tile([128, 1024], BF16, tag="h1", name=f"h1_{t}")
            br_auto(h1[:], p[:], wt["t1d2"][:])
            h1t[t] = h1

        def seg512(c0, width):
            j = c0
            while j < c0 + width:
                j1 = min((j // 512 + 1) * 512, c0 + width)
                yield j, j1
                j = j1

        def unit_p2(sub):
            (s, g, gp, aoff, wsub, segs) = sub
            h1s = h1t[gp]
            p = psum.tile([128, 1024], F32, tag="ps", space="PSUM",
                          name=f"p2_{s}_{g}_{aoff}")[:, 0:wsub]
            for (half, lo, ln, doff) in segs:
                wst = wt["w2e"] if half == 0 else wt["w2o"]
                for j, j1 in seg512(doff, ln):
                    nc.tensor.matmul(p[:, j:j1], wst[:],
                                     h1s[:, lo + j - doff : lo + j1 - doff],
                                     start=True, stop=True)
            h2 = h2p.tile([128, 1024], BF16, tag="h2",
                          name=f"h2_{s}_{g}_{aoff}")[:, 0:wsub]
            br_auto(h2, p, wt["t2"][:])
            return h2

        def unit_p3(sub, h2_ap, half):
            (s, g, gp, aoff, wsub, segs) = sub
            wst = wt["w3a"] if half == 0 else wt["w3b"]
            bias = wt["t3a"] if half == 0 else wt["t3b"]
            base = g * CHUNK + aoff
            acc = (accA if half == 0 else accB)[:, base : base + wsub]
            p = psum.tile([128, 1024], F32, tag="ps", space="PSUM",
                          name=f"p3{half}_{s}_{g}_{aoff}")[:, 0:wsub]
            for j, j1 in seg512(0, wsub):
                nc.tensor.matmul(p[:, j:j1], wst[:], h2_ap[:, j:j1],
                                 start=True, stop=True)
            if s == 0:
                br_auto(acc, p, bias[:])
            else:
                tt = tmp.tile([128, 1024], BF16, tag="tt",
                              name=f"tt{half}_{s}_{g}_{aoff}")[:, 0:wsub]
                br_auto(tt, p, bias[:])
                debt["dve"] += (wsub / 2.0 + 151.0) / 0.96 + 160.0
                nc.vector.tensor_max(acc, tt, acc)

        def unit_fin(g):
            for j in (0, 1024):
                p = psum.tile([128, 1024], F32, tag="ps", space="PSUM",
                              name=f"pc_{g}_{j}")
                for j2 in (0, 512):
                    for q in range(2):
                        cols = (g + q) * CHUNK + j + j2
                        dst = p[64 * q : 64 * q + 64, j2 : j2 + 512]
                        nc.tensor.matmul(dst, wt["wc0"][:],
                                         accA[:, cols : cols + 512],
                                         start=True, stop=False)
                        nc.tensor.matmul(dst, wt["wc1"][:],
                                         accB[:, cols : cols + 512],
                                         start=False, stop=True)
                sc = scp.tile([128, 1024], BF16, tag="sc", name=f"sc_{g}_{j}")
                br_auto(sc[:], p[:], wt["bc2"][:])
                for q in range(2):
                    o = (g + q) * CHUNK + j
                    nc.gpsimd.dma_start(out=comp[:, o : o + 1024],
                                        in_=sc[64 * q : 64 * q + 64, :])

        # ---- software-pipelined emission over sub-items ----
        subs = _sub_items(items)
        last_sub = {}
        for i, sub in enumerate(subs):
            last_sub[sub[1]] = i  # last sub index per chunk
        done = [False] * G
        finned = [False] * G

        def try_fins(g):
            done[g] = True
            gp = g - (g % 2)
            if done[gp] and done[gp + 1] and not finned[gp]:
                finned[gp] = True
                unit_fin(gp)

        have = {sub[1] for sub in subs}
        for g in range(G):
            if g not in have:
                done[g] = True
        for g in range(0, G, 2):
            if done[g] and done[g + 1]:
                finned[g] = True

        p2q = list(enumerate(subs))
        p3q = []

        def pump(grp_done_upto):
            while p3q:
                i, sub, h2a = p3q.pop(0)
                unit_p3(sub, h2a, 0)
                unit_p3(sub, h2a, 1)
                if i == last_sub[sub[1]]:
                    try_fins(sub[1])
            while p2q and p2q[0][1][2] < grp_done_upto:
                i, sub = p2q.pop(0)
                p3q.append((i, sub, unit_p2(sub)))

        for t in range(G1):
            unit_p1(t)
            pump(t)
        pump(G1)
        pump(G1)

    nc.compile()
    return nc


# ------------------------------------------------------------------ driver

_CACHE = {}


def kernel(voxels, coors, batch_size, w1, g1, b1, m1, v1,
           w2, g2, b2, m2, v2, w3, g3, b3, m3, v3, wc, bc,
           _trace=False):
    voxels = np.asarray(voxels, np.float32)
    coors = np.asarray(coors, np.int32)
    plan_key, items, G1, vox_all, rows_all = _build_plan_and_data(
        voxels, coors)
    G = plan_key[0]

    if plan_key not in _CACHE:
        _CACHE[plan_key] = build_program(G, items, G1)
    nc = _CACHE[plan_key]

    folded = _fold_weights(
        np.asarray(w1, np.float32), np.asarray(g1, np.float32),
        np.asarray(b1, np.float32), np.asarray(m1, np.float32),
        np.asarray(v1, np.float32),
        np.asarray(w2, np.float32), np.asarray(g2, np.float32),
        np.asarray(b2, np.float32), np.asarray(m2, np.float32),
        np.asarray(v2, np.float32),
        np.asarray(w3, np.float32), np.asarray(g3, np.float32),
        np.asarray(b3, np.float32), np.asarray(m3, np.float32),
        np.asarray(v3, np.float32),
        np.asarray(wc, np.float32), np.asarray(bc, np.float32))

    import ml_dtypes
    in_maps = [
        {"vox": vox_all[c].astype(ml_dtypes.bfloat16), **folded}
        for c in range(N_CORES)
    ]
    res = bass_utils.run_bass_kernel_spmd(
        nc, in_maps, core_ids=list(range(N_CORES)), trace=_trace)

    out = np.zeros((B, 64, GX * GY), np.float32)
    for c in range(N_CORES):
        cm = np.asarray(res.results[c]["comp"]).astype(np.float32)
        rows = rows_all[c]
        real = rows != PAD
        gcell = rows[real] + c * CELLS_PER_CORE
        b_core = c // (N_CORES // B)
        xy = gcell - b_core * (GX * GY)
        out[b_core][:, xy] = cm[:, real]
    out = out.reshape(B, 64, GX, GY)
    if _trace:
        return out, res
    return out


# revision 26
# speedup vs baseline: 1.5119x; 1.0001x over previous
"""BevFeatureEncoder on 8 Trainium2 NeuronCores.

Strategy (data-parallel over BEV grid slabs):
  - The 2*480*360 BEV cells are split into 8 contiguous ranges of 43200
    cells; points are routed on host to the core owning their cell, so
    the segment_max reduction is fully local to each core.
  - Per core, occupied cells are sorted by point count DESC and grouped
    into chunks of 2048 cells. Slot s of a chunk covers only the prefix
    of cells with count > s (widths shrink with s, rounded to 256), so
    there is no power-of-2 slot padding. Items run in s-major order so
    the per-chunk max-accumulate chains never serialize back-to-back.
  - BN scale/shift is folded into the weights ON HOST (numpy): the
    device sees pre-folded bf16 stationaries + per-partition biases and
    has no weight-prep preamble.
  - Device dataflow: 2048-wide units, each 4 matmuls into a 4-bank PSUM
    tile + ONE wide drain (relu+bias) on ACT or DVE, debt-balanced.
    Layer 1 packs two cells per column (block-diag [8,128]); layer 2
    unpacks via even/odd zero-padded stationaries; the compression runs
    col-tiled (out partitions 0:64 / 64:128 for a chunk pair). Max
    accumulation (slots s>0) is split: ACT/DVE relu-drain to a temp
    tile, then a 2x-rate bf16 SBUF tensor_tensor max into the
    accumulator planes.
  - Output is compacted [64, G*2048] bf16; host places real columns
    into the zeroed [B, C, GX, GY] grid. Chunk structure is equalized
    across cores (max widths) so one SPMD program serves all 8 cores.
"""

import numpy as np

import concourse.bacc as bacc
import concourse.bass as bass
import concourse.mybir as mybir
import concourse.tile as tile
from concourse import bass_utils

GX, GY = 480, 360
B = 2
EPS = 1e-5
N_CORES = 8
CELLS_PER_CORE = (B * GX * GY) // N_CORES  # 43200
CHUNK = 2048  # cells per chunk (fin granularity; 4 PSUM banks wide)
PAD = -1

F32 = mybir.dt.float32
BF16 = mybir.dt.bfloat16

Relu = mybir.ActivationFunctionType.Relu


# ---------------------------------------------------------------- host prep


def _build_plan_and_data(voxels, coors):
    """Route points to cores; build the equalized slot plan and per-core
    device inputs."""
    seg = (
        coors[:, 0].astype(np.int64) * (GX * GY)
        + coors[:, 1].astype(np.int64) * GY
        + coors[:, 2].astype(np.int64)
    )
    core_of = seg // CELLS_PER_CORE

    per_core = []
    for c in range(N_CORES):
        idx = np.nonzero(core_of == c)[0]
        seg_local = seg[idx] - c * CELLS_PER_CORE
        order = np.argsort(seg_local, kind="stable")
        seg_sorted = seg_local[order]
        cells, starts, counts = np.unique(
            seg_sorted, return_index=True, return_counts=True
        )
        o2 = np.argsort(-counts, kind="stable")
        cells, starts, counts = cells[o2], starts[o2], counts[o2]
        pts = idx[order]
        per_core.append((cells, starts, counts, pts))

    n_occ_max = max(len(pc[0]) for pc in per_core)
    G = -(-n_occ_max // CHUNK)
    if G % 2:
        G += 1  # fin works on chunk pairs
    kmax = int(max(pc[2].max() for pc in per_core))

    widths = np.zeros((kmax, G), np.int64)
    for cells, starts, counts, pts in per_core:
        n = len(cells)
        for g in range(G):
            lo, hi = g * CHUNK, min((g + 1) * CHUNK, n)
            if lo >= hi:
                continue
            cg = counts[lo:hi]
            for s in range(int(cg[0])):
                widths[s, g] = max(widths[s, g], int((cg > s).sum()))
    widths = np.minimum(-(-widths // 128) * 128, CHUNK)

    # items in s-major order; greedy-pack h1 blocks (w//2 cols) into
    # 2048-col groups so every item's h1 is contiguous in one tile
    order = []
    for s in range(kmax):
        for g in range(G):
            if widths[s, g] > 0:
                key = (s * 0.14, g) if g <= 1 else (float(s), g)
                order.append((key, s, g))
    order.sort()
    items = []  # (s, g, w, grp, off)
    grp, off = 0, 0
    if True:
        for _, s, g in order:
            w = int(widths[s, g])
            h = w // 2
            if off + h > 1024:
                grp += 1
                off = 0
            items.append((s, g, w, grp, off))
            off += h
    G1 = grp + 1

    vox_all = np.zeros((N_CORES, 8, G1 * 1024), np.float32)
    rows_all = np.full((N_CORES, G * CHUNK), PAD, np.int64)

    for core, (cells, starts, counts, pts) in enumerate(per_core):
        n = len(cells)
        rows_all[core, :n] = cells
        for (s, g, w, gp, of) in items:
            lo = g * CHUNK
            ncell = max(0, min(w, n - lo))
            if ncell == 0:
                continue
            cnt = counts[lo : lo + ncell]
            so = np.minimum(s, cnt - 1)
            p_idx = pts[starts[lo : lo + ncell] + so]
            feats = np.zeros((w, 4), np.float32)
            feats[:ncell] = voxels[p_idx]
            h = w // 2
            packed = np.concatenate([feats[:h].T, feats[h:].T], axis=0)
            pc0 = gp * 1024 + of
            vox_all[core, :, pc0 : pc0 + h] = packed
    return (G, kmax, tuple(widths.flatten().tolist())), items, G1, \
        vox_all, rows_all


def _fold_weights(w1, g1, b1, m1, v1, w2, g2, b2, m2, v2,
                  w3, g3, b3, m3, v3, wc, bc):
    """Fold BN (eval) into the linear weights, build device layouts."""
    import ml_dtypes

    def sb(g, b, m, v):
        s = g / np.sqrt(v + EPS)
        return s.astype(np.float32), (b - m * s).astype(np.float32)

    s1, t1 = sb(g1, b1, m1, v1)
    s2, t2 = sb(g2, b2, m2, v2)
    s3, t3 = sb(g3, b3, m3, v3)
    w1f = (w1 * s1).astype(np.float32)  # [4,64]
    w2f = (w2 * s2).astype(np.float32)  # [64,128]
    w3f = (w3 * s3).astype(np.float32)  # [128,256]

    bf = ml_dtypes.bfloat16
    w1d8 = np.zeros((8, 128), np.float32)
    w1d8[0:4, 0:64] = w1f
    w1d8[4:8, 64:128] = w1f
    w2e = np.zeros((128, 128), np.float32)
    w2e[0:64] = w2f
    w2o = np.zeros((128, 128), np.float32)
    w2o[64:128] = w2f
    out = {
        "w1d8": w1d8.astype(bf),
        "w2e": w2e.astype(bf),
        "w2o": w2o.astype(bf),
        "w3a": np.ascontiguousarray(w3f[:, 0:128]).astype(bf),
        "w3b": np.ascontiguousarray(w3f[:, 128:256]).astype(bf),
        "wc0": np.ascontiguousarray(wc[0:128]).astype(np.float32).astype(bf),
        "wc1": np.ascontiguousarray(wc[128:256]).astype(np.float32).astype(bf),
        "t1d2": np.concatenate([t1, t1])[:, None].astype(np.float32),
        "t2": t2[:, None].astype(np.float32),
        "t3a": t3[0:128, None].astype(np.float32),
        "t3b": t3[128:256, None].astype(np.float32),
        "bc2": np.concatenate([bc, bc])[:, None].astype(np.float32),
    }
    return out


# ------------------------------------------------------------- bass program


def _sub_items(items):
    """Split each item into sub-items of <= 1024 h2 cols.

    Sub-item: (s, g, acc_off, wsub, segs) where segs is a list of
    (half, h1_lo, h1_len, dst_off) mapping h1 ranges (half 0 = even
    rows 0:64, 1 = odd rows 64:128) to the sub-item's h2 cols.
    """
    subs = []
    for (s, g, w, gp, of) in items:
        h = w // 2
        # concatenated h2 col space: [0:h) even, [h:w) odd
        u = 0
        while u * 1024 < w:
            lo, hi = u * 1024, min((u + 1) * 1024, w)
            segs = []
            for half, base in ((0, 0), (1, h)):
                a = max(lo, base)
                b = min(hi, base + h)
                if a < b:
                    segs.append((half, of + a - base, b - a, a - lo))
            subs.append((s, g, gp, u * 1024, hi - lo, segs))
            u += 1
    return subs


def build_program(G, items, G1):
    nc = bacc.Bacc("TRN2", target_bir_lowering=False, debug=False,
                   num_devices=N_CORES)

    vox = nc.dram_tensor("vox", [8, G1 * 1024], BF16,
                         kind="ExternalInput").ap()
    wdram = {}
    for name, shape, dt in [
        ("w1d8", [8, 128], BF16), ("w2e", [128, 128], BF16),
        ("w2o", [128, 128], BF16),
        ("w3a", [128, 128], BF16), ("w3b", [128, 128], BF16),
        ("wc0", [128, 64], BF16), ("wc1", [128, 64], BF16),
        ("t1d2", [128, 1], F32), ("t2", [128, 1], F32),
        ("t3a", [128, 1], F32), ("t3b", [128, 1], F32),
        ("bc2", [128, 1], F32),
    ]:
        wdram[name] = (nc.dram_tensor(name, shape, dt,
                                      kind="ExternalInput").ap(), shape, dt)
    comp = nc.dram_tensor("comp", [64, G * CHUNK], BF16,
                          kind="ExternalOutput").ap()

    from contextlib import ExitStack
    with tile.TileContext(nc) as tc, ExitStack() as ctx:
        cpool = ctx.enter_context(tc.tile_pool(name="const", bufs=1))

        _q = [nc.scalar, nc.gpsimd, nc.sync]
        wt = {}
        for i, name in enumerate(["w1d8", "t1d2", "w2e", "w2o", "t2",
                                  "w3a", "w3b", "t3a", "t3b",
                                  "wc0", "wc1", "bc2"]):
            ap, shape, dt = wdram[name]
            t = cpool.tile(shape, dt, tag=name)
            _q[i % 3].dma_start(out=t[:], in_=ap)
            wt[name] = t

        vox_sb = cpool.tile([8, G1 * 1024], BF16)
        # many small DMAs in consumption order, spread across queues
        bounds, c = [0], 0
        for p in [1024, 1024, 1024, 2048, 2048] + [4096] * 64:
            if c >= G1 * 1024:
                break
            c = min(c + p, G1 * 1024)
            bounds.append(c)
        for i in range(len(bounds) - 1):
            _q[i % 3].dma_start(out=vox_sb[:, bounds[i]:bounds[i + 1]],
                                in_=vox[:, bounds[i]:bounds[i + 1]])

        accA = cpool.tile([128, G * CHUNK], BF16)
        accB = cpool.tile([128, G * CHUNK], BF16)
        w0 = {}
        for (s, g, w, gp, of) in items:
            w0[g] = max(w0.get(g, 0), w)
        for g in range(G):
            wg = w0.get(g, 0)
            if wg < CHUNK:
                nc.vector.memset(accA[:, g * CHUNK + wg : (g + 1) * CHUNK], 0.0)
                nc.vector.memset(accB[:, g * CHUNK + wg : (g + 1) * CHUNK], 0.0)

        h1p = ctx.enter_context(tc.tile_pool(name="h1p", bufs=8))
        h2p = ctx.enter_context(tc.tile_pool(name="h2p", bufs=8))
        tmp = ctx.enter_context(tc.tile_pool(name="tmp", bufs=6))
        scp = ctx.enter_context(tc.tile_pool(name="scp", bufs=4))
        psum = ctx.enter_context(tc.tile_pool(name="psum", bufs=4,
                                              space="PSUM"))

        debt = {"act": 0.0, "dve": 0.0}
        COST = {
            "act": lambda n: (n + 352.0) / 1.33 + 250.0,
            "dve": lambda n: (n + 120.0) / 0.94 + 160.0,
        }

        def br_auto(out_ap, in_ap, bias_ap):
            n = in_ap.shape[-1]
            eng = min(("act", "dve"), key=lambda e: debt[e] + COST[e](n))
            debt[eng] += COST[eng](n)
            if eng == "act":
                nc.scalar.activation(out_ap, in_ap, Relu, bias=bias_ap,
                                     scale=1.0)
            else:
                nc.vector.tensor_scalar(out_ap, in_ap, bias_ap, 0.0,
                                        op0=mybir.AluOpType.add,
                                        op1=mybir.AluOpType.max)

        h1t = {}

        def unit_p1(t):
            p = psum.tile([128, 1024], F32, tag="ps", space="PSUM",
                          name=f"p1_{t}")
            slab = vox_sb[:, t * 1024 : (t + 1) * 1024]
            for i in range(2):
                nc.tensor.matmul(p[:, i * 512 : (i + 1) * 512],
                                 wt["w1d8"][:],
                                 slab[:, i * 512 : (i + 1) * 512],
                                 start=True, stop=True)
            h1 = h1p.tile([128, 1024], BF16, tag="h1", name=f"h1_{t}")
            br_auto(h1[:], p[:], wt["t1d2"][:])
            h1t[t] = h1

        def seg512(c0, width):
            j = c0
            while j < c0 + width:
                j1 = min((j // 512 + 1) * 512, c0 + width)
                yield j, j1
                j = j1

        def unit_p2(sub):
            (s, g, gp, aoff, wsub, segs) = sub
            h1s = h1t[gp]
            p = psum.tile([128, 1024], F32, tag="ps", space="PSUM",
                          name=f"p2_{s}_{g}_{aoff}")[:, 0:wsub]
            for (half, lo, ln, doff) in segs:
                wst = wt["w2e"] if half == 0 else wt["w2o"]
                for j, j1 in seg512(doff, ln):
                    nc.tensor.matmul(p[:, j:j1], wst[:],
                                     h1s[:, lo + j - doff : lo + j1 - doff],
                                     start=True, stop=True)
            h2 = h2p.tile([128, 1024], BF16, tag="h2",
                          name=f"h2_{s}_{g}_{aoff}")[:, 0:wsub]
            br_auto(h2, p, wt["t2"][:])
            return h2

        def unit_p3(sub, h2_ap, half):
            (s, g, gp, aoff, wsub, segs) = sub
            wst = wt["w3a"] if half == 0 else wt["w3b"]
            bias = wt["t3a"] if half == 0 else wt["t3b"]
            base = g * CHUNK + aoff
            acc = (accA if half == 0 else accB)[:, base : base + wsub]
            p = psum.tile([128, 1024], F32, tag="ps", space="PSUM",
                          name=f"p3{half}_{s}_{g}_{aoff}")[:, 0:wsub]
            for j, j1 in seg512(0, wsub):
                nc.tensor.matmul(p[:, j:j1], wst[:], h2_ap[:, j:j1],
                                 start=True, stop=True)
            if s == 0:
                br_auto(acc, p, bias[:])
            else:
                tt = tmp.tile([128, 1024], BF16, tag="tt",
                              name=f"tt{half}_{s}_{g}_{aoff}")[:, 0:wsub]
                br_auto(tt, p, bias[:])
                debt["dve"] += (wsub / 2.0 + 151.0) / 0.96 + 160.0
                nc.vector.tensor_max(acc, tt, acc)

        def unit_fin(g):
            for j in (0, 1024):
                p = psum.tile([128, 1024], F32, tag="ps", space="PSUM",
                              name=f"pc_{g}_{j}")
                for j2 in (0, 512):
                    for q in range(2):
                        cols = (g + q) * CHUNK + j + j2
                        dst = p[64 * q : 64 * q + 64, j2 : j2 + 512]
                        nc.tensor.matmul(dst, wt["wc0"][:],
                                         accA[:, cols : cols + 512],
                                         start=True, stop=False)
                        nc.tensor.matmul(dst, wt["wc1"][:],
                                         accB[:, cols : cols + 512],
                                         start=False, stop=True)
                sc = scp.tile([128, 1024], BF16, tag="sc", name=f"sc_{g}_{j}")
                br_auto(sc[:], p[:], wt["bc2"][:])
                for q in range(2):
                    o = (g + q) * CHUNK + j
                    nc.gpsimd.dma_start(out=comp[:, o : o + 1024],
                                        in_=sc[64 * q : 64 * q + 64, :])

        # ---- software-pipelined emission over sub-items ----
        subs = _sub_items(items)
        last_sub = {}
        for i, sub in enumerate(subs):
            last_sub[sub[1]] = i  # last sub index per chunk
        done = [False] * G
        finned = [False] * G

        def try_fins(g):
            done[g] = True
            gp = g - (g % 2)
            if done[gp] and done[gp + 1] and not finned[gp]:
                finned[gp] = True
                unit_fin(gp)

        have = {sub[1] for sub in subs}
        for g in range(G):
            if g not in have:
                done[g] = True
        for g in range(0, G, 2):
            if done[g] and done[g + 1]:
                finned[g] = True

        p2q = list(enumerate(subs))
        p3q = []

        def pump(grp_done_upto):
            while p3q:
                i, sub, h2a = p3q.pop(0)
                unit_p3(sub, h2a, 0)
                unit_p3(sub, h2a, 1)
                if i == last_sub[sub[1]]:
                    try_fins(sub[1])
            while p2q and p2q[0][1][2] < grp_done_upto:
                i, sub = p2q.pop(0)
                p3q.append((i, sub, unit_p2(sub)))

        for t in range(G1):
            unit_p1(t)
            pump(t)
        pump(G1)
        pump(G1)

    nc.compile()
    return nc


# ------------------------------------------------------------------ driver

_CACHE = {}


def kernel(voxels, coors, batch_size, w1, g1, b1, m1, v1,
           w2, g2, b2, m2, v2, w3, g3, b3, m3, v3, wc, bc,
           _trace=False):
    voxels = np.asarray(voxels, np.float32)
    coors = np.asarray(coors, np.int32)
    plan_key, items, G1, vox_all, rows_all = _build_plan_and_data(
        voxels, coors)
    G = plan_key[0]

    if plan_key not in _CACHE:
        _CACHE[plan_key] = build_program(G, items, G1)
    nc = _CACHE[plan_key]

    folded = _fold_weights(
        np.asarray(w1, np.float32), np.asarray(g1, np.float32),
        np.asarray(b1, np.float32), np.asarray(m1, np.float32),
        np.asarray(v1, np.float32),
        np.asarray(w2, np.float32), np.asarray(g2, np.float32),
        np.asarray(b2, np.float32), np.asarray(m2, np.float32),
        np.asarray(v2, np.float32),
        np.asarray(w3, np.float32), np.asarray(g3, np.float32),
        np.asarray(b3, np.float32), np.asarray(m3, np.float32),
        np.asarray(v3, np.float32),
        np.asarray(wc, np.float32), np.asarray(bc, np.float32))

    import ml_dtypes
    in_maps = [
        {"vox": vox_all[c].astype(ml_dtypes.bfloat16), **folded}
        for c in range(N_CORES)
    ]
    res = bass_utils.run_bass_kernel_spmd(
        nc, in_maps, core_ids=list(range(N_CORES)), trace=_trace)

    out = np.zeros((B, 64, GX * GY), np.float32)
    for c in range(N_CORES):
        cm = np.asarray(res.results[c]["comp"]).astype(np.float32)
        rows = rows_all[c]
        real = rows != PAD
        gcell = rows[real] + c * CELLS_PER_CORE
        b_core = c // (N_CORES // B)
        xy = gcell - b_core * (GX * GY)
        out[b_core][:, xy] = cm[:, real]
    out = out.reshape(B, 64, GX, GY)
    if _trace:
        return out, res
    return out


# revision 27
# speedup vs baseline: 1.5147x; 1.0018x over previous
"""BevFeatureEncoder on 8 Trainium2 NeuronCores.

Strategy (data-parallel over BEV grid slabs):
  - The 2*480*360 BEV cells are split into 8 contiguous ranges of 43200
    cells; points are routed on host to the core owning their cell, so
    the segment_max reduction is fully local to each core.
  - Per core, occupied cells are sorted by point count DESC and grouped
    into chunks of 2048 cells. Slot s of a chunk covers only the prefix
    of cells with count > s (widths shrink with s, rounded to 256), so
    there is no power-of-2 slot padding. Items run in s-major order so
    the per-chunk max-accumulate chains never serialize back-to-back.
  - BN scale/shift is folded into the weights ON HOST (numpy): the
    device sees pre-folded bf16 stationaries + per-partition biases and
    has no weight-prep preamble.
  - Device dataflow: 2048-wide units, each 4 matmuls into a 4-bank PSUM
    tile + ONE wide drain (relu+bias) on ACT or DVE, debt-balanced.
    Layer 1 packs two cells per column (block-diag [8,128]); layer 2
    unpacks via even/odd zero-padded stationaries; the compression runs
    col-tiled (out partitions 0:64 / 64:128 for a chunk pair). Max
    accumulation (slots s>0) is split: ACT/DVE relu-drain to a temp
    tile, then a 2x-rate bf16 SBUF tensor_tensor max into the
    accumulator planes.
  - Output is compacted [64, G*2048] bf16; host places real columns
    into the zeroed [B, C, GX, GY] grid. Chunk structure is equalized
    across cores (max widths) so one SPMD program serves all 8 cores.
"""

import numpy as np

import concourse.bacc as bacc
import concourse.bass as bass
import concourse.mybir as mybir
import concourse.tile as tile
from concourse import bass_utils

GX, GY = 480, 360
B = 2
EPS = 1e-5
N_CORES = 8
CELLS_PER_CORE = (B * GX * GY) // N_CORES  # 43200
CHUNK = 2048  # cells per chunk (fin granularity; 4 PSUM banks wide)
PAD = -1

F32 = mybir.dt.float32
BF16 = mybir.dt.bfloat16

Relu = mybir.ActivationFunctionType.Relu


# ---------------------------------------------------------------- host prep


def _build_plan_and_data(voxels, coors):
    """Route points to cores; build the equalized slot plan and per-core
    device inputs."""
    seg = (
        coors[:, 0].astype(np.int64) * (GX * GY)
        + coors[:, 1].astype(np.int64) * GY
        + coors[:, 2].astype(np.int64)
    )
    core_of = seg // CELLS_PER_CORE

    per_core = []
    for c in range(N_CORES):
        idx = np.nonzero(core_of == c)[0]
        seg_local = seg[idx] - c * CELLS_PER_CORE
        order = np.argsort(seg_local, kind="stable")
        seg_sorted = seg_local[order]
        cells, starts, counts = np.unique(
            seg_sorted, return_index=True, return_counts=True
        )
        o2 = np.argsort(-counts, kind="stable")
        cells, starts, counts = cells[o2], starts[o2], counts[o2]
        pts = idx[order]
        per_core.append((cells, starts, counts, pts))

    n_occ_max = max(len(pc[0]) for pc in per_core)
    G = -(-n_occ_max // CHUNK)
    if G % 2:
        G += 1  # fin works on chunk pairs
    kmax = int(max(pc[2].max() for pc in per_core))

    widths = np.zeros((kmax, G), np.int64)
    for cells, starts, counts, pts in per_core:
        n = len(cells)
        for g in range(G):
            lo, hi = g * CHUNK, min((g + 1) * CHUNK, n)
            if lo >= hi:
                continue
            cg = counts[lo:hi]
            for s in range(int(cg[0])):
                widths[s, g] = max(widths[s, g], int((cg > s).sum()))
    widths = np.minimum(-(-widths // 128) * 128, CHUNK)

    # items in s-major order; greedy-pack h1 blocks (w//2 cols) into
    # 2048-col groups so every item's h1 is contiguous in one tile
    order = []
    for s in range(kmax):
        for g in range(G):
            if widths[s, g] > 0:
                key = (s * 0.14, g) if g <= 3 else (float(s), g)
                order.append((key, s, g))
    order.sort()
    items = []  # (s, g, w, grp, off)
    grp, off = 0, 0
    if True:
        for _, s, g in order:
            w = int(widths[s, g])
            h = w // 2
            if off + h > 1024:
                grp += 1
                off = 0
            items.append((s, g, w, grp, off))
            off += h
    G1 = grp + 1

    vox_all = np.zeros((N_CORES, 8, G1 * 1024), np.float32)
    rows_all = np.full((N_CORES, G * CHUNK), PAD, np.int64)

    for core, (cells, starts, counts, pts) in enumerate(per_core):
        n = len(cells)
        rows_all[core, :n] = cells
        for (s, g, w, gp, of) in items:
            lo = g * CHUNK
            ncell = max(0, min(w, n - lo))
            if ncell == 0:
                continue
            cnt = counts[lo : lo + ncell]
            so = np.minimum(s, cnt - 1)
            p_idx = pts[starts[lo : lo + ncell] + so]
            feats = np.zeros((w, 4), np.float32)
            feats[:ncell] = voxels[p_idx]
            h = w // 2
            packed = np.concatenate([feats[:h].T, feats[h:].T], axis=0)
            pc0 = gp * 1024 + of
            vox_all[core, :, pc0 : pc0 + h] = packed
    return (G, kmax, tuple(widths.flatten().tolist())), items, G1, \
        vox_all, rows_all


def _fold_weights(w1, g1, b1, m1, v1, w2, g2, b2, m2, v2,
                  w3, g3, b3, m3, v3, wc, bc):
    """Fold BN (eval) into the linear weights, build device layouts."""
    import ml_dtypes

    def sb(g, b, m, v):
        s = g / np.sqrt(v + EPS)
        return s.astype(np.float32), (b - m * s).astype(np.float32)

    s1, t1 = sb(g1, b1, m1, v1)
    s2, t2 = sb(g2, b2, m2, v2)
    s3, t3 = sb(g3, b3, m3, v3)
    w1f = (w1 * s1).astype(np.float32)  # [4,64]
    w2f = (w2 * s2).astype(np.float32)  # [64,128]
    w3f = (w3 * s3).astype(np.float32)  # [128,256]

    bf = ml_dtypes.bfloat16
    w1d8 = np.zeros((8, 128), np.float32)
    w1d8[0:4, 0:64] = w1f
    w1d8[4:8, 64:128] = w1f
    w2e = np.zeros((128, 128), np.float32)
    w2e[0:64] = w2f
    w2o = np.zeros((128, 128), np.float32)
    w2o[64:128] = w2f
    out = {
        "w1d8": w1d8.astype(bf),
        "w2e": w2e.astype(bf),
        "w2o": w2o.astype(bf),
        "w3a": np.ascontiguousarray(w3f[:, 0:128]).astype(bf),
        "w3b": np.ascontiguousarray(w3f[:, 128:256]).astype(bf),
        "wc0": np.ascontiguousarray(wc[0:128]).astype(np.float32).astype(bf),
        "wc1": np.ascontiguousarray(wc[128:256]).astype(np.float32).astype(bf),
        "t1d2": np.concatenate([t1, t1])[:, None].astype(np.float32),
        "t2": t2[:, None].astype(np.float32),
        "t3a": t3[0:128, None].astype(np.float32),
        "t3b": t3[128:256, None].astype(np.float32),
        "bc2": np.concatenate([bc, bc])[:, None].astype(np.float32),
    }
    return out


# ------------------------------------------------------------- bass program


def _sub_items(items):
    """Split each item into sub-items of <= 1024 h2 cols.

    Sub-item: (s, g, acc_off, wsub, segs) where segs is a list of
    (half, h1_lo, h1_len, dst_off) mapping h1 ranges (half 0 = even
    rows 0:64, 1 = odd rows 64:128) to the sub-item's h2 cols.
    """
    subs = []
    for (s, g, w, gp, of) in items:
        h = w // 2
        # concatenated h2 col space: [0:h) even, [h:w) odd
        u = 0
        while u * 1024 < w:
            lo, hi = u * 1024, min((u + 1) * 1024, w)
            segs = []
            for half, base in ((0, 0), (1, h)):
                a = max(lo, base)
                b = min(hi, base + h)
                if a < b:
                    segs.append((half, of + a - base, b - a, a - lo))
            subs.append((s, g, gp, u * 1024, hi - lo, segs))
            u += 1
    return subs


def build_program(G, items, G1):
    nc = bacc.Bacc("TRN2", target_bir_lowering=False, debug=False,
                   num_devices=N_CORES)

    vox = nc.dram_tensor("vox", [8, G1 * 1024], BF16,
                         kind="ExternalInput").ap()
    wdram = {}
    for name, shape, dt in [
        ("w1d8", [8, 128], BF16), ("w2e", [128, 128], BF16),
        ("w2o", [128, 128], BF16),
        ("w3a", [128, 128], BF16), ("w3b", [128, 128], BF16),
        ("wc0", [128, 64], BF16), ("wc1", [128, 64], BF16),
        ("t1d2", [128, 1], F32), ("t2", [128, 1], F32),
        ("t3a", [128, 1], F32), ("t3b", [128, 1], F32),
        ("bc2", [128, 1], F32),
    ]:
        wdram[name] = (nc.dram_tensor(name, shape, dt,
                                      kind="ExternalInput").ap(), shape, dt)
    comp = nc.dram_tensor("comp", [64, G * CHUNK], BF16,
                          kind="ExternalOutput").ap()

    from contextlib import ExitStack
    with tile.TileContext(nc) as tc, ExitStack() as ctx:
        cpool = ctx.enter_context(tc.tile_pool(name="const", bufs=1))

        _q = [nc.scalar, nc.gpsimd, nc.sync]
        wt = {}
        for i, name in enumerate(["w1d8", "t1d2", "w2e", "w2o", "t2",
                                  "w3a", "w3b", "t3a", "t3b",
                                  "wc0", "wc1", "bc2"]):
            ap, shape, dt = wdram[name]
            t = cpool.tile(shape, dt, tag=name)
            _q[i % 3].dma_start(out=t[:], in_=ap)
            wt[name] = t

        vox_sb = cpool.tile([8, G1 * 1024], BF16)
        # many small DMAs in consumption order, spread across queues
        bounds, c = [0], 0
        for p in [1024, 1024, 1024, 2048, 2048] + [4096] * 64:
            if c >= G1 * 1024:
                break
            c = min(c + p, G1 * 1024)
            bounds.append(c)
        for i in range(len(bounds) - 1):
            _q[i % 3].dma_start(out=vox_sb[:, bounds[i]:bounds[i + 1]],
                                in_=vox[:, bounds[i]:bounds[i + 1]])

        accA = cpool.tile([128, G * CHUNK], BF16)
        accB = cpool.tile([128, G * CHUNK], BF16)
        w0 = {}
        for (s, g, w, gp, of) in items:
            w0[g] = max(w0.get(g, 0), w)
        for g in range(G):
            wg = w0.get(g, 0)
            if wg < CHUNK:
                nc.vector.memset(accA[:, g * CHUNK + wg : (g + 1) * CHUNK], 0.0)
                nc.vector.memset(accB[:, g * CHUNK + wg : (g + 1) * CHUNK], 0.0)

        h1p = ctx.enter_context(tc.tile_pool(name="h1p", bufs=8))
        h2p = ctx.enter_context(tc.tile_pool(name="h2p", bufs=8))
        tmp = ctx.enter_context(tc.tile_pool(name="tmp", bufs=6))
        scp = ctx.enter_context(tc.tile_pool(name="scp", bufs=4))
        psum = ctx.enter_context(tc.tile_pool(name="psum", bufs=4,
                                              space="PSUM"))

        debt = {"act": 0.0, "dve": 0.0}
        COST = {
            "act": lambda n: (n + 352.0) / 1.33 + 250.0,
            "dve": lambda n: (n + 120.0) / 0.94 + 160.0,
        }

        def br_auto(out_ap, in_ap, bias_ap):
            n = in_ap.shape[-1]
            eng = min(("act", "dve"), key=lambda e: debt[e] + COST[e](n))
            debt[eng] += COST[eng](n)
            if eng == "act":
                nc.scalar.activation(out_ap, in_ap, Relu, bias=bias_ap,
                                     scale=1.0)
            else:
                nc.vector.tensor_scalar(out_ap, in_ap, bias_ap, 0.0,
                                        op0=mybir.AluOpType.add,
                                        op1=mybir.AluOpType.max)

        h1t = {}

        def unit_p1(t):
            p = psum.tile([128, 1024], F32, tag="ps", space="PSUM",
                          name=f"p1_{t}")
            slab = vox_sb[:, t * 1024 : (t + 1) * 1024]
            for i in range(2):
                nc.tensor.matmul(p[:, i * 512 : (i + 1) * 512],
                                 wt["w1d8"][:],
                                 slab[:, i * 512 : (i + 1) * 512],
                                 start=True, stop=True)
            h1 = h1p.tile([128, 1024], BF16, tag="h1", name=f"h1_{t}")
            br_auto(h1[:], p[:], wt["t1d2"][:])
            h1t[t] = h1

        def seg512(c0, width):
            j = c0
            while j < c0 + width:
                j1 = min((j // 512 + 1) * 512, c0 + width)
                yield j, j1
                j = j1

        def unit_p2(sub):
            (s, g, gp, aoff, wsub, segs) = sub
            h1s = h1t[gp]
            p = psum.tile([128, 1024], F32, tag="ps", space="PSUM",
                          name=f"p2_{s}_{g}_{aoff}")[:, 0:wsub]
            for (half, lo, ln, doff) in segs:
                wst = wt["w2e"] if half == 0 else wt["w2o"]
                for j, j1 in seg512(doff, ln):
                    nc.tensor.matmul(p[:, j:j1], wst[:],
                                     h1s[:, lo + j - doff : lo + j1 - doff],
                                     start=True, stop=True)
            h2 = h2p.tile([128, 1024], BF16, tag="h2",
                          name=f"h2_{s}_{g}_{aoff}")[:, 0:wsub]
            br_auto(h2, p, wt["t2"][:])
            return h2

        def unit_p3(sub, h2_ap, half):
            (s, g, gp, aoff, wsub, segs) = sub
            wst = wt["w3a"] if half == 0 else wt["w3b"]
            bias = wt["t3a"] if half == 0 else wt["t3b"]
            base = g * CHUNK + aoff
            acc = (accA if half == 0 else accB)[:, base : base + wsub]
            p = psum.tile([128, 1024], F32, tag="ps", space="PSUM",
                          name=f"p3{half}_{s}_{g}_{aoff}")[:, 0:wsub]
            for j, j1 in seg512(0, wsub):
                nc.tensor.matmul(p[:, j:j1], wst[:], h2_ap[:, j:j1],
                                 start=True, stop=True)
            if s == 0:
                br_auto(acc, p, bias[:])
            else:
                tt = tmp.tile([128, 1024], BF16, tag="tt",
                              name=f"tt{half}_{s}_{g}_{aoff}")[:, 0:wsub]
                br_auto(tt, p, bias[:])
                debt["dve"] += (wsub / 2.0 + 151.0) / 0.96 + 160.0
                nc.vector.tensor_max(acc, tt, acc)

        def unit_fin(g):
            for j in (0, 1024):
                p = psum.tile([128, 1024], F32, tag="ps", space="PSUM",
                              name=f"pc_{g}_{j}")
                for j2 in (0, 512):
                    for q in range(2):
                        cols = (g + q) * CHUNK + j + j2
                        dst = p[64 * q : 64 * q + 64, j2 : j2 + 512]
                        nc.tensor.matmul(dst, wt["wc0"][:],
                                         accA[:, cols : cols + 512],
                                         start=True, stop=False)
                        nc.tensor.matmul(dst, wt["wc1"][:],
                                         accB[:, cols : cols + 512],
                                         start=False, stop=True)
                sc = scp.tile([128, 1024], BF16, tag="sc", name=f"sc_{g}_{j}")
                br_auto(sc[:], p[:], wt["bc2"][:])
                for q in range(2):
                    o = (g + q) * CHUNK + j
                    nc.gpsimd.dma_start(out=comp[:, o : o + 1024],
                                        in_=sc[64 * q : 64 * q + 64, :])

        # ---- software-pipelined emission over sub-items ----
        subs = _sub_items(items)
        last_sub = {}
        for i, sub in enumerate(subs):
            last_sub[sub[1]] = i  # last sub index per chunk
        done = [False] * G
        finned = [False] * G

        def try_fins(g):
            done[g] = True
            gp = g - (g % 2)
            if done[gp] and done[gp + 1] and not finned[gp]:
                finned[gp] = True
                unit_fin(gp)

        have = {sub[1] for sub in subs}
        for g in range(G):
            if g not in have:
                done[g] = True
        for g in range(0, G, 2):
            if done[g] and done[g + 1]:
                finned[g] = True

        p2q = list(enumerate(subs))
        p3q = []

        def pump(grp_done_upto):
            while p3q:
                i, sub, h2a = p3q.pop(0)
                unit_p3(sub, h2a, 0)
                unit_p3(sub, h2a, 1)
                if i == last_sub[sub[1]]:
                    try_fins(sub[1])
            while p2q and p2q[0][1][2] < grp_done_upto:
                i, sub = p2q.pop(0)
                p3q.append((i, sub, unit_p2(sub)))

        for t in range(G1):
            unit_p1(t)
            pump(t)
        pump(G1)
        pump(G1)

    nc.compile()
    return nc


# ------------------------------------------------------------------ driver

_CACHE = {}


def kernel(voxels, coors, batch_size, w1, g1, b1, m1, v1,
           w2, g2, b2, m2, v2, w3, g3, b3, m3, v3, wc, bc,
           _trace=False):
    voxels = np.asarray(voxels, np.float32)
    coors = np.asarray(coors, np.int32)
    plan_key, items, G1, vox_all, rows_all = _build_plan_and_data(
        voxels, coors)
    G = plan_key[0]

    if plan_key not in _CACHE:
        _CACHE[plan_key] = build_program(G, items, G1)
    nc = _CACHE[plan_key]

    folded = _fold_weights(
        np.asarray(w1, np.float32), np.asarray(g1, np.float32),
        np.asarray(b1, np.float32), np.asarray(m1, np.float32),
        np.asarray(v1, np.float32),
        np.asarray(w2, np.float32), np.asarray(g2, np.float32),
        np.asarray(b2, np.float32), np.asarray(m2, np.float32),
        np.asarray(v2, np.float32),
        np.asarray(w3, np.float32), np.asarray(g3, np.float32),
        np.asarray(b3, np.float32), np.asarray(m3, np.float32),
        np.asarray(v3, np.float32),
        np.asarray(wc, np.float32), np.asarray(bc, np.float32))

    import ml_dtypes
    in_maps = [
        {"vox": vox_all[c].astype(ml_dtypes.bfloat16), **folded}
        for c in range(N_CORES)
    ]
    res = bass_utils.run_bass_kernel_spmd(
        nc, in_maps, core_ids=list(range(N_CORES)), trace=_trace)

    out = np.zeros((B, 64, GX * GY), np.float32)
    for c in range(N_CORES):
        cm = np.asarray(res.results[c]["comp"]).astype(np.float32)
        rows = rows_all[c]
        real = rows != PAD
        gcell = rows[real] + c * CELLS_PER_CORE
        b_core = c // (N_CORES // B)
        xy = gcell - b_core * (GX * GY)
        out[b_core][:, xy] = cm[:, real]
    out = out.reshape(B, 64, GX, GY)
    if _trace:
        return out, res
    return out


# revision 30
# speedup vs baseline: 1.5626x; 1.0316x over previous
"""BevFeatureEncoder on 8 Trainium2 NeuronCores.

Strategy (data-parallel over BEV grid slabs):
  - The 2*480*360 BEV cells are split into 8 contiguous ranges of 43200
    cells; points are routed on host to the core owning their cell, so
    the segment_max reduction is fully local to each core.
  - Per core, occupied cells are sorted by point count DESC and grouped
    into chunks of 2048 cells. Slot s of a chunk covers only the prefix
    of cells with count > s (widths shrink with s, rounded to 256), so
    there is no power-of-2 slot padding. Items run in s-major order so
    the per-chunk max-accumulate chains never serialize back-to-back.
  - BN scale/shift is folded into the weights ON HOST (numpy): the
    device sees pre-folded bf16 stationaries + per-partition biases and
    has no weight-prep preamble.
  - Device dataflow: 2048-wide units, each 4 matmuls into a 4-bank PSUM
    tile + ONE wide drain (relu+bias) on ACT or DVE, debt-balanced.
    Layer 1 packs two cells per column (block-diag [8,128]); layer 2
    unpacks via even/odd zero-padded stationaries; the compression runs
    col-tiled (out partitions 0:64 / 64:128 for a chunk pair). Max
    accumulation (slots s>0) is split: ACT/DVE relu-drain to a temp
    tile, then a 2x-rate bf16 SBUF tensor_tensor max into the
    accumulator planes.
  - Output is compacted [64, G*2048] bf16; host places real columns
    into the zeroed [B, C, GX, GY] grid. Chunk structure is equalized
    across cores (max widths) so one SPMD program serves all 8 cores.
"""

import numpy as np

import concourse.bacc as bacc
import concourse.bass as bass
import concourse.mybir as mybir
import concourse.tile as tile
from concourse import bass_utils

GX, GY = 480, 360
B = 2
EPS = 1e-5
N_CORES = 8
CELLS_PER_CORE = (B * GX * GY) // N_CORES  # 43200
CHUNK = 2048  # cells per chunk (fin granularity; 4 PSUM banks wide)
PAD = -1

F32 = mybir.dt.float32
BF16 = mybir.dt.bfloat16

Relu = mybir.ActivationFunctionType.Relu


# ---------------------------------------------------------------- host prep


def _build_plan_and_data(voxels, coors):
    """Route points to cores; build the equalized slot plan and per-core
    device inputs."""
    seg = (
        coors[:, 0].astype(np.int64) * (GX * GY)
        + coors[:, 1].astype(np.int64) * GY
        + coors[:, 2].astype(np.int64)
    )
    core_of = seg // CELLS_PER_CORE

    per_core = []
    for c in range(N_CORES):
        idx = np.nonzero(core_of == c)[0]
        seg_local = seg[idx] - c * CELLS_PER_CORE
        order = np.argsort(seg_local, kind="stable")
        seg_sorted = seg_local[order]
        cells, starts, counts = np.unique(
            seg_sorted, return_index=True, return_counts=True
        )
        o2 = np.argsort(-counts, kind="stable")
        cells, starts, counts = cells[o2], starts[o2], counts[o2]
        pts = idx[order]
        per_core.append((cells, starts, counts, pts))

    n_occ_max = max(len(pc[0]) for pc in per_core)
    G = -(-n_occ_max // CHUNK)
    if G % 2:
        G += 1  # fin works on chunk pairs
    kmax = int(max(pc[2].max() for pc in per_core))

    widths = np.zeros((kmax, G), np.int64)
    for cells, starts, counts, pts in per_core:
        n = len(cells)
        for g in range(G):
            lo, hi = g * CHUNK, min((g + 1) * CHUNK, n)
            if lo >= hi:
                continue
            cg = counts[lo:hi]
            for s in range(int(cg[0])):
                widths[s, g] = max(widths[s, g], int((cg > s).sum()))
    widths = np.minimum(-(-widths // 128) * 128, CHUNK)

    # items in s-major order; greedy-pack h1 blocks (w//2 cols) into
    # 2048-col groups so every item's h1 is contiguous in one tile
    order = []
    for s in range(kmax):
        for g in range(G):
            if widths[s, g] > 0:
                key = (s * 0.14, g) if g <= 3 else (float(s), g)
                order.append((key, s, g))
    order.sort()
    items = []  # (s, g, w, grp, off)
    grp, off = 0, 0
    if True:
        for _, s, g in order:
            w = int(widths[s, g])
            h = w // 2
            if off + h > 1024:
                grp += 1
                off = 0
            items.append((s, g, w, grp, off))
            off += h
    G1 = grp + 1

    vox_all = np.zeros((N_CORES, 8, G1 * 1024), np.float32)
    rows_all = np.full((N_CORES, G * CHUNK), PAD, np.int64)

    for core, (cells, starts, counts, pts) in enumerate(per_core):
        n = len(cells)
        rows_all[core, :n] = cells
        for (s, g, w, gp, of) in items:
            lo = g * CHUNK
            ncell = max(0, min(w, n - lo))
            if ncell == 0:
                continue
            cnt = counts[lo : lo + ncell]
            so = np.minimum(s, cnt - 1)
            p_idx = pts[starts[lo : lo + ncell] + so]
            feats = np.zeros((w, 4), np.float32)
            feats[:ncell] = voxels[p_idx]
            h = w // 2
            packed = np.concatenate([feats[:h].T, feats[h:].T], axis=0)
            pc0 = gp * 1024 + of
            vox_all[core, :, pc0 : pc0 + h] = packed
    return (G, kmax, tuple(widths.flatten().tolist())), items, G1, \
        vox_all, rows_all


def _fold_weights(w1, g1, b1, m1, v1, w2, g2, b2, m2, v2,
                  w3, g3, b3, m3, v3, wc, bc):
    """Fold BN (eval) into the linear weights, build device layouts."""
    import ml_dtypes

    def sb(g, b, m, v):
        s = g / np.sqrt(v + EPS)
        return s.astype(np.float32), (b - m * s).astype(np.float32)

    s1, t1 = sb(g1, b1, m1, v1)
    s2, t2 = sb(g2, b2, m2, v2)
    s3, t3 = sb(g3, b3, m3, v3)
    w1f = (w1 * s1).astype(np.float32)  # [4,64]
    w2f = (w2 * s2).astype(np.float32)  # [64,128]
    w3f = (w3 * s3).astype(np.float32)  # [128,256]

    bf = ml_dtypes.bfloat16
    w1d8 = np.zeros((8, 128), np.float32)
    w1d8[0:4, 0:64] = w1f
    w1d8[4:8, 64:128] = w1f
    w2e = np.zeros((128, 128), np.float32)
    w2e[0:64] = w2f
    w2o = np.zeros((128, 128), np.float32)
    w2o[64:128] = w2f
    out = {
        "w1d8": w1d8.astype(bf),
        "w2e": w2e.astype(bf),
        "w2o": w2o.astype(bf),
        "w3a": np.ascontiguousarray(w3f[:, 0:128]).astype(bf),
        "w3b": np.ascontiguousarray(w3f[:, 128:256]).astype(bf),
        "wc0": np.ascontiguousarray(wc[0:128]).astype(np.float32).astype(bf),
        "wc1": np.ascontiguousarray(wc[128:256]).astype(np.float32).astype(bf),
        "t1d2": np.concatenate([t1, t1])[:, None].astype(np.float32),
        "t2": t2[:, None].astype(np.float32),
        "t3a": t3[0:128, None].astype(np.float32),
        "t3b": t3[128:256, None].astype(np.float32),
        "bc2": np.concatenate([bc, bc])[:, None].astype(np.float32),
    }
    return out


# ------------------------------------------------------------- bass program


def _sub_items(items):
    """Split each item into sub-items of <= 1024 h2 cols.

    Sub-item: (s, g, acc_off, wsub, segs) where segs is a list of
    (half, h1_lo, h1_len, dst_off) mapping h1 ranges (half 0 = even
    rows 0:64, 1 = odd rows 64:128) to the sub-item's h2 cols.
    """
    subs = []
    for (s, g, w, gp, of) in items:
        h = w // 2
        # concatenated h2 col space: [0:h) even, [h:w) odd
        u = 0
        while u * 1024 < w:
            lo, hi = u * 1024, min((u + 1) * 1024, w)
            segs = []
            for half, base in ((0, 0), (1, h)):
                a = max(lo, base)
                b = min(hi, base + h)
                if a < b:
                    segs.append((half, of + a - base, b - a, a - lo))
            subs.append((s, g, gp, u * 1024, hi - lo, segs))
            u += 1
    return subs


def build_program(G, items, G1):
    nc = bacc.Bacc("TRN2", target_bir_lowering=False, debug=False,
                   num_devices=N_CORES)

    vox = nc.dram_tensor("vox", [8, G1 * 1024], BF16,
                         kind="ExternalInput").ap()
    wdram = {}
    for name, shape, dt in [
        ("w1d8", [8, 128], BF16), ("w2e", [128, 128], BF16),
        ("w2o", [128, 128], BF16),
        ("w3a", [128, 128], BF16), ("w3b", [128, 128], BF16),
        ("wc0", [128, 64], BF16), ("wc1", [128, 64], BF16),
        ("t1d2", [128, 1], F32), ("t2", [128, 1], F32),
        ("t3a", [128, 1], F32), ("t3b", [128, 1], F32),
        ("bc2", [128, 1], F32),
    ]:
        wdram[name] = (nc.dram_tensor(name, shape, dt,
                                      kind="ExternalInput").ap(), shape, dt)
    comp = nc.dram_tensor("comp", [64, G * CHUNK], BF16,
                          kind="ExternalOutput").ap()

    from contextlib import ExitStack
    with tile.TileContext(nc) as tc, ExitStack() as ctx:
        cpool = ctx.enter_context(tc.tile_pool(name="const", bufs=1))

        _q = [nc.scalar, nc.gpsimd, nc.sync]
        wt = {}
        for i, name in enumerate(["w1d8", "t1d2", "w2e", "w2o", "t2",
                                  "w3a", "w3b", "t3a", "t3b",
                                  "wc0", "wc1", "bc2"]):
            ap, shape, dt = wdram[name]
            t = cpool.tile(shape, dt, tag=name)
            _q[i % 3].dma_start(out=t[:], in_=ap)
            wt[name] = t

        vox_sb = cpool.tile([8, G1 * 1024], BF16)
        # vox pieces are issued lazily from the emission loop (see
        # issue_vox) so early p1 units do not wait on later pieces
        vox_issued = [0]

        def issue_vox(upto_grp):
            while vox_issued[0] < min(upto_grp, G1):
                t = vox_issued[0]
                (nc.sync if t % 2 == 0 else nc.gpsimd).dma_start(
                    out=vox_sb[:, t * 1024 : (t + 1) * 1024],
                    in_=vox[:, t * 1024 : (t + 1) * 1024])
                vox_issued[0] += 1

        accA = cpool.tile([128, G * CHUNK], BF16)
        accB = cpool.tile([128, G * CHUNK], BF16)
        w0 = {}
        for (s, g, w, gp, of) in items:
            w0[g] = max(w0.get(g, 0), w)
        for g in range(G):
            wg = w0.get(g, 0)
            if wg < CHUNK:
                nc.vector.memset(accA[:, g * CHUNK + wg : (g + 1) * CHUNK], 0.0)
                nc.vector.memset(accB[:, g * CHUNK + wg : (g + 1) * CHUNK], 0.0)

        h1p = ctx.enter_context(tc.tile_pool(name="h1p", bufs=8))
        h2p = ctx.enter_context(tc.tile_pool(name="h2p", bufs=8))
        tmp = ctx.enter_context(tc.tile_pool(name="tmp", bufs=6))
        scp = ctx.enter_context(tc.tile_pool(name="scp", bufs=4))
        psum = ctx.enter_context(tc.tile_pool(name="psum", bufs=4,
                                              space="PSUM"))

        debt = {"act": 0.0, "dve": 0.0}
        COST = {
            "act": lambda n: (n + 352.0) / 1.33 + 250.0,
            "dve": lambda n: (n + 120.0) / 0.94 + 160.0,
        }

        def br_auto(out_ap, in_ap, bias_ap):
            n = in_ap.shape[-1]
            eng = min(("act", "dve"), key=lambda e: debt[e] + COST[e](n))
            debt[eng] += COST[eng](n)
            if eng == "act":
                nc.scalar.activation(out_ap, in_ap, Relu, bias=bias_ap,
                                     scale=1.0)
            else:
                nc.vector.tensor_scalar(out_ap, in_ap, bias_ap, 0.0,
                                        op0=mybir.AluOpType.add,
                                        op1=mybir.AluOpType.max)

        h1t = {}

        def unit_p1(t):
            p = psum.tile([128, 1024], F32, tag="ps", space="PSUM",
                          name=f"p1_{t}")
            slab = vox_sb[:, t * 1024 : (t + 1) * 1024]
            for i in range(2):
                nc.tensor.matmul(p[:, i * 512 : (i + 1) * 512],
                                 wt["w1d8"][:],
                                 slab[:, i * 512 : (i + 1) * 512],
                                 start=True, stop=True)
            h1 = h1p.tile([128, 1024], BF16, tag="h1", name=f"h1_{t}")
            br_auto(h1[:], p[:], wt["t1d2"][:])
            h1t[t] = h1

        def seg512(c0, width):
            j = c0
            while j < c0 + width:
                j1 = min((j // 512 + 1) * 512, c0 + width)
                yield j, j1
                j = j1

        def unit_p2(sub):
            (s, g, gp, aoff, wsub, segs) = sub
            h1s = h1t[gp]
            p = psum.tile([128, 1024], F32, tag="ps", space="PSUM",
                          name=f"p2_{s}_{g}_{aoff}")[:, 0:wsub]
            for (half, lo, ln, doff) in segs:
                wst = wt["w2e"] if half == 0 else wt["w2o"]
                for j, j1 in seg512(doff, ln):
                    nc.tensor.matmul(p[:, j:j1], wst[:],
                                     h1s[:, lo + j - doff : lo + j1 - doff],
                                     start=True, stop=True)
            h2 = h2p.tile([128, 1024], BF16, tag="h2",
                          name=f"h2_{s}_{g}_{aoff}")[:, 0:wsub]
            br_auto(h2, p, wt["t2"][:])
            return h2

        def unit_p3(sub, h2_ap, half):
            (s, g, gp, aoff, wsub, segs) = sub
            wst = wt["w3a"] if half == 0 else wt["w3b"]
            bias = wt["t3a"] if half == 0 else wt["t3b"]
            base = g * CHUNK + aoff
            acc = (accA if half == 0 else accB)[:, base : base + wsub]
            p = psum.tile([128, 1024], F32, tag="ps", space="PSUM",
                          name=f"p3{half}_{s}_{g}_{aoff}")[:, 0:wsub]
            for j, j1 in seg512(0, wsub):
                nc.tensor.matmul(p[:, j:j1], wst[:], h2_ap[:, j:j1],
                                 start=True, stop=True)
            if s == 0:
                br_auto(acc, p, bias[:])
            else:
                tt = tmp.tile([128, 1024], BF16, tag="tt",
                              name=f"tt{half}_{s}_{g}_{aoff}")[:, 0:wsub]
                br_auto(tt, p, bias[:])
                debt["dve"] += (wsub / 2.0 + 151.0) / 0.96 + 160.0
                nc.vector.tensor_max(acc, tt, acc)

        def unit_fin(g):
            for j in (0, 1024):
                p = psum.tile([128, 1024], F32, tag="ps", space="PSUM",
                              name=f"pc_{g}_{j}")
                for j2 in (0, 512):
                    for q in range(2):
                        cols = (g + q) * CHUNK + j + j2
                        dst = p[64 * q : 64 * q + 64, j2 : j2 + 512]
                        nc.tensor.matmul(dst, wt["wc0"][:],
                                         accA[:, cols : cols + 512],
                                         start=True, stop=False)
                        nc.tensor.matmul(dst, wt["wc1"][:],
                                         accB[:, cols : cols + 512],
                                         start=False, stop=True)
                sc = scp.tile([128, 1024], BF16, tag="sc", name=f"sc_{g}_{j}")
                br_auto(sc[:], p[:], wt["bc2"][:])
                for q in range(2):
                    o = (g + q) * CHUNK + j
                    nc.gpsimd.dma_start(out=comp[:, o : o + 1024],
                                        in_=sc[64 * q : 64 * q + 64, :])

        # ---- software-pipelined emission over sub-items ----
        subs = _sub_items(items)
        last_sub = {}
        for i, sub in enumerate(subs):
            last_sub[sub[1]] = i  # last sub index per chunk
        done = [False] * G
        finned = [False] * G

        def try_fins(g):
            done[g] = True
            gp = g - (g % 2)
            if done[gp] and done[gp + 1] and not finned[gp]:
                finned[gp] = True
                unit_fin(gp)

        have = {sub[1] for sub in subs}
        for g in range(G):
            if g not in have:
                done[g] = True
        for g in range(0, G, 2):
            if done[g] and done[g + 1]:
                finned[g] = True

        p2q = list(enumerate(subs))
        p3q = []

        def pump(grp_done_upto):
            while p3q:
                i, sub, h2a = p3q.pop(0)
                unit_p3(sub, h2a, 0)
                unit_p3(sub, h2a, 1)
                if i == last_sub[sub[1]]:
                    try_fins(sub[1])
            while p2q and p2q[0][1][2] < grp_done_upto:
                i, sub = p2q.pop(0)
                p3q.append((i, sub, unit_p2(sub)))

        issue_vox(2)
        for t in range(G1):
            issue_vox(t + 3)
            unit_p1(t)
            pump(t)
        pump(G1)
        pump(G1)

    nc.compile()
    return nc


# ------------------------------------------------------------------ driver

_CACHE = {}


def kernel(voxels, coors, batch_size, w1, g1, b1, m1, v1,
           w2, g2, b2, m2, v2, w3, g3, b3, m3, v3, wc, bc,
           _trace=False):
    voxels = np.asarray(voxels, np.float32)
    coors = np.asarray(coors, np.int32)
    plan_key, items, G1, vox_all, rows_all = _build_plan_and_data(
        voxels, coors)
    G = plan_key[0]

    if plan_key not in _CACHE:
        _CACHE[plan_key] = build_program(G, items, G1)
    nc = _CACHE[plan_key]

    folded = _fold_weights(
        np.asarray(w1, np.float32), np.asarray(g1, np.float32),
        np.asarray(b1, np.float32), np.asarray(m1, np.float32),
        np.asarray(v1, np.float32),
        np.asarray(w2, np.float32), np.asarray(g2, np.float32),
        np.asarray(b2, np.float32), np.asarray(m2, np.float32),
        np.asarray(v2, np.float32),
        np.asarray(w3, np.float32), np.asarray(g3, np.float32),
        np.asarray(b3, np.float32), np.asarray(m3, np.float32),
        np.asarray(v3, np.float32),
        np.asarray(wc, np.float32), np.asarray(bc, np.float32))

    import ml_dtypes
    in_maps = [
        {"vox": vox_all[c].astype(ml_dtypes.bfloat16), **folded}
        for c in range(N_CORES)
    ]
    res = bass_utils.run_bass_kernel_spmd(
        nc, in_maps, core_ids=list(range(N_CORES)), trace=_trace)

    out = np.zeros((B, 64, GX * GY), np.float32)
    for c in range(N_CORES):
        cm = np.asarray(res.results[c]["comp"]).astype(np.float32)
        rows = rows_all[c]
        real = rows != PAD
        gcell = rows[real] + c * CELLS_PER_CORE
        b_core = c // (N_CORES // B)
        xy = gcell - b_core * (GX * GY)
        out[b_core][:, xy] = cm[:, real]
    out = out.reshape(B, 64, GX, GY)
    if _trace:
        return out, res
    return out


# revision 31
# speedup vs baseline: 1.5973x; 1.0222x over previous
"""BevFeatureEncoder on 8 Trainium2 NeuronCores.

Strategy (data-parallel over BEV grid slabs):
  - The 2*480*360 BEV cells are split into 8 contiguous ranges of 43200
    cells; points are routed on host to the core owning their cell, so
    the segment_max reduction is fully local to each core.
  - Per core, occupied cells are sorted by point count DESC and grouped
    into chunks of 2048 cells. Slot s of a chunk covers only the prefix
    of cells with count > s (widths shrink with s, rounded to 256), so
    there is no power-of-2 slot padding. Items run in s-major order so
    the per-chunk max-accumulate chains never serialize back-to-back.
  - BN scale/shift is folded into the weights ON HOST (numpy): the
    device sees pre-folded bf16 stationaries + per-partition biases and
    has no weight-prep preamble.
  - Device dataflow: 2048-wide units, each 4 matmuls into a 4-bank PSUM
    tile + ONE wide drain (relu+bias) on ACT or DVE, debt-balanced.
    Layer 1 packs two cells per column (block-diag [8,128]); layer 2
    unpacks via even/odd zero-padded stationaries; the compression runs
    col-tiled (out partitions 0:64 / 64:128 for a chunk pair). Max
    accumulation (slots s>0) is split: ACT/DVE relu-drain to a temp
    tile, then a 2x-rate bf16 SBUF tensor_tensor max into the
    accumulator planes.
  - Output is compacted [64, G*2048] bf16; host places real columns
    into the zeroed [B, C, GX, GY] grid. Chunk structure is equalized
    across cores (max widths) so one SPMD program serves all 8 cores.
"""

import numpy as np

import concourse.bacc as bacc
import concourse.bass as bass
import concourse.mybir as mybir
import concourse.tile as tile
from concourse import bass_utils

GX, GY = 480, 360
B = 2
EPS = 1e-5
N_CORES = 8
CELLS_PER_CORE = (B * GX * GY) // N_CORES  # 43200
CHUNK = 2048  # cells per chunk (fin granularity; 4 PSUM banks wide)
PAD = -1

F32 = mybir.dt.float32
BF16 = mybir.dt.bfloat16

Relu = mybir.ActivationFunctionType.Relu


# ---------------------------------------------------------------- host prep


def _build_plan_and_data(voxels, coors):
    """Route points to cores; build the equalized slot plan and per-core
    device inputs."""
    seg = (
        coors[:, 0].astype(np.int64) * (GX * GY)
        + coors[:, 1].astype(np.int64) * GY
        + coors[:, 2].astype(np.int64)
    )
    core_of = seg // CELLS_PER_CORE

    per_core = []
    for c in range(N_CORES):
        idx = np.nonzero(core_of == c)[0]
        seg_local = seg[idx] - c * CELLS_PER_CORE
        order = np.argsort(seg_local, kind="stable")
        seg_sorted = seg_local[order]
        cells, starts, counts = np.unique(
            seg_sorted, return_index=True, return_counts=True
        )
        o2 = np.argsort(-counts, kind="stable")
        cells, starts, counts = cells[o2], starts[o2], counts[o2]
        pts = idx[order]
        per_core.append((cells, starts, counts, pts))

    n_occ_max = max(len(pc[0]) for pc in per_core)
    G = -(-n_occ_max // CHUNK)
    if G % 2:
        G += 1  # fin works on chunk pairs
    kmax = int(max(pc[2].max() for pc in per_core))

    widths = np.zeros((kmax, G), np.int64)
    for cells, starts, counts, pts in per_core:
        n = len(cells)
        for g in range(G):
            lo, hi = g * CHUNK, min((g + 1) * CHUNK, n)
            if lo >= hi:
                continue
            cg = counts[lo:hi]
            for s in range(int(cg[0])):
                widths[s, g] = max(widths[s, g], int((cg > s).sum()))
    widths = np.minimum(-(-widths // 128) * 128, CHUNK)

    # items in s-major order; greedy-pack h1 blocks (w//2 cols) into
    # 2048-col groups so every item's h1 is contiguous in one tile
    order = []
    for s in range(kmax):
        for g in range(G):
            if widths[s, g] > 0:
                key = (s * 0.14, g) if g <= 3 else (float(s), g)
                order.append((key, s, g))
    order.sort()
    items = []  # (s, g, w, grp, off)
    grp, off = 0, 0
    if True:
        for _, s, g in order:
            w = int(widths[s, g])
            h = w // 2
            if off + h > 1024:
                grp += 1
                off = 0
            items.append((s, g, w, grp, off))
            off += h
    G1 = grp + 1

    vox_all = np.zeros((N_CORES, 8, G1 * 1024), np.float32)
    rows_all = np.full((N_CORES, G * CHUNK), PAD, np.int64)

    for core, (cells, starts, counts, pts) in enumerate(per_core):
        n = len(cells)
        rows_all[core, :n] = cells
        for (s, g, w, gp, of) in items:
            lo = g * CHUNK
            ncell = max(0, min(w, n - lo))
            if ncell == 0:
                continue
            cnt = counts[lo : lo + ncell]
            so = np.minimum(s, cnt - 1)
            p_idx = pts[starts[lo : lo + ncell] + so]
            feats = np.zeros((w, 4), np.float32)
            feats[:ncell] = voxels[p_idx]
            h = w // 2
            packed = np.concatenate([feats[:h].T, feats[h:].T], axis=0)
            pc0 = gp * 1024 + of
            vox_all[core, :, pc0 : pc0 + h] = packed
    return (G, kmax, tuple(widths.flatten().tolist())), items, G1, \
        vox_all, rows_all


def _fold_weights(w1, g1, b1, m1, v1, w2, g2, b2, m2, v2,
                  w3, g3, b3, m3, v3, wc, bc):
    """Fold BN (eval) into the linear weights, build device layouts."""
    import ml_dtypes

    def sb(g, b, m, v):
        s = g / np.sqrt(v + EPS)
        return s.astype(np.float32), (b - m * s).astype(np.float32)

    s1, t1 = sb(g1, b1, m1, v1)
    s2, t2 = sb(g2, b2, m2, v2)
    s3, t3 = sb(g3, b3, m3, v3)
    w1f = (w1 * s1).astype(np.float32)  # [4,64]
    w2f = (w2 * s2).astype(np.float32)  # [64,128]
    w3f = (w3 * s3).astype(np.float32)  # [128,256]

    bf = ml_dtypes.bfloat16
    w1d8 = np.zeros((8, 128), np.float32)
    w1d8[0:4, 0:64] = w1f
    w1d8[4:8, 64:128] = w1f
    w2e = np.zeros((128, 128), np.float32)
    w2e[0:64] = w2f
    w2o = np.zeros((128, 128), np.float32)
    w2o[64:128] = w2f
    out = {
        "w1d8": w1d8.astype(bf),
        "w2e": w2e.astype(bf),
        "w2o": w2o.astype(bf),
        "w3a": np.ascontiguousarray(w3f[:, 0:128]).astype(bf),
        "w3b": np.ascontiguousarray(w3f[:, 128:256]).astype(bf),
        "wc0": np.ascontiguousarray(wc[0:128]).astype(np.float32).astype(bf),
        "wc1": np.ascontiguousarray(wc[128:256]).astype(np.float32).astype(bf),
        "t1d2": np.concatenate([t1, t1])[:, None].astype(np.float32),
        "t2": t2[:, None].astype(np.float32),
        "t3a": t3[0:128, None].astype(np.float32),
        "t3b": t3[128:256, None].astype(np.float32),
        "bc2": np.concatenate([bc, bc])[:, None].astype(np.float32),
    }
    return out


# ------------------------------------------------------------- bass program


def _sub_items(items):
    """Split each item into sub-items of <= 1024 h2 cols.

    Sub-item: (s, g, acc_off, wsub, segs) where segs is a list of
    (half, h1_lo, h1_len, dst_off) mapping h1 ranges (half 0 = even
    rows 0:64, 1 = odd rows 64:128) to the sub-item's h2 cols.
    """
    subs = []
    for (s, g, w, gp, of) in items:
        h = w // 2
        # concatenated h2 col space: [0:h) even, [h:w) odd
        u = 0
        while u * 1024 < w:
            lo, hi = u * 1024, min((u + 1) * 1024, w)
            segs = []
            for half, base in ((0, 0), (1, h)):
                a = max(lo, base)
                b = min(hi, base + h)
                if a < b:
                    segs.append((half, of + a - base, b - a, a - lo))
            subs.append((s, g, gp, u * 1024, hi - lo, segs))
            u += 1
    return subs


def build_program(G, items, G1):
    nc = bacc.Bacc("TRN2", target_bir_lowering=False, debug=False,
                   num_devices=N_CORES)

    vox = nc.dram_tensor("vox", [8, G1 * 1024], BF16,
                         kind="ExternalInput").ap()
    wdram = {}
    for name, shape, dt in [
        ("w1d8", [8, 128], BF16), ("w2e", [128, 128], BF16),
        ("w2o", [128, 128], BF16),
        ("w3a", [128, 128], BF16), ("w3b", [128, 128], BF16),
        ("wc0", [128, 64], BF16), ("wc1", [128, 64], BF16),
        ("t1d2", [128, 1], F32), ("t2", [128, 1], F32),
        ("t3a", [128, 1], F32), ("t3b", [128, 1], F32),
        ("bc2", [128, 1], F32),
    ]:
        wdram[name] = (nc.dram_tensor(name, shape, dt,
                                      kind="ExternalInput").ap(), shape, dt)
    comp = nc.dram_tensor("comp", [64, G * CHUNK], BF16,
                          kind="ExternalOutput").ap()

    from contextlib import ExitStack
    with tile.TileContext(nc) as tc, ExitStack() as ctx:
        cpool = ctx.enter_context(tc.tile_pool(name="const", bufs=1))

        _q = [nc.scalar, nc.gpsimd, nc.sync]
        vox_sb = cpool.tile([8, G1 * 1024], BF16)
        # vox pieces are issued lazily from the emission loop (see
        # issue_vox) so early p1 units do not wait on later pieces
        vox_issued = [0]

        def issue_vox(upto_grp):
            while vox_issued[0] < min(upto_grp, G1):
                t = vox_issued[0]
                (nc.sync if t % 2 == 0 else nc.gpsimd).dma_start(
                    out=vox_sb[:, t * 1024 : (t + 1) * 1024],
                    in_=vox[:, t * 1024 : (t + 1) * 1024])
                vox_issued[0] += 1

        wt = {}

        def wload(i, name):
            ap, shape, dt = wdram[name]
            t = cpool.tile(shape, dt, tag=name)
            _q[i % 3].dma_start(out=t[:], in_=ap)
            wt[name] = t

        wload(0, "w1d8")
        issue_vox(2)
        for i, name in enumerate(["t1d2", "w2e", "w2o", "t2",
                                  "w3a", "w3b", "t3a", "t3b",
                                  "wc0", "wc1", "bc2"], start=1):
            wload(i, name)

        accA = cpool.tile([128, G * CHUNK], BF16)
        accB = cpool.tile([128, G * CHUNK], BF16)
        w0 = {}
        for (s, g, w, gp, of) in items:
            w0[g] = max(w0.get(g, 0), w)
        for g in range(G):
            wg = w0.get(g, 0)
            if wg < CHUNK:
                nc.vector.memset(accA[:, g * CHUNK + wg : (g + 1) * CHUNK], 0.0)
                nc.vector.memset(accB[:, g * CHUNK + wg : (g + 1) * CHUNK], 0.0)

        h1p = ctx.enter_context(tc.tile_pool(name="h1p", bufs=8))
        h2p = ctx.enter_context(tc.tile_pool(name="h2p", bufs=8))
        tmp = ctx.enter_context(tc.tile_pool(name="tmp", bufs=6))
        scp = ctx.enter_context(tc.tile_pool(name="scp", bufs=4))
        psum = ctx.enter_context(tc.tile_pool(name="psum", bufs=4,
                                              space="PSUM"))

        debt = {"act": 0.0, "dve": 0.0}
        COST = {
            "act": lambda n: (n + 352.0) / 1.33 + 250.0,
            "dve": lambda n: (n + 120.0) / 0.94 + 160.0,
        }

        def br_auto(out_ap, in_ap, bias_ap):
            n = in_ap.shape[-1]
            eng = min(("act", "dve"), key=lambda e: debt[e] + COST[e](n))
            debt[eng] += COST[eng](n)
            if eng == "act":
                nc.scalar.activation(out_ap, in_ap, Relu, bias=bias_ap,
                                     scale=1.0)
            else:
                nc.vector.tensor_scalar(out_ap, in_ap, bias_ap, 0.0,
                                        op0=mybir.AluOpType.add,
                                        op1=mybir.AluOpType.max)

        h1t = {}

        def unit_p1(t):
            p = psum.tile([128, 1024], F32, tag="ps", space="PSUM",
                          name=f"p1_{t}")
            slab = vox_sb[:, t * 1024 : (t + 1) * 1024]
            for i in range(2):
                nc.tensor.matmul(p[:, i * 512 : (i + 1) * 512],
                                 wt["w1d8"][:],
                                 slab[:, i * 512 : (i + 1) * 512],
                                 start=True, stop=True)
            h1 = h1p.tile([128, 1024], BF16, tag="h1", name=f"h1_{t}")
            br_auto(h1[:], p[:], wt["t1d2"][:])
            h1t[t] = h1

        def seg512(c0, width):
            j = c0
            while j < c0 + width:
                j1 = min((j // 512 + 1) * 512, c0 + width)
                yield j, j1
                j = j1

        def unit_p2(sub):
            (s, g, gp, aoff, wsub, segs) = sub
            h1s = h1t[gp]
            p = psum.tile([128, 1024], F32, tag="ps", space="PSUM",
                          name=f"p2_{s}_{g}_{aoff}")[:, 0:wsub]
            for (half, lo, ln, doff) in segs:
                wst = wt["w2e"] if half == 0 else wt["w2o"]
                for j, j1 in seg512(doff, ln):
                    nc.tensor.matmul(p[:, j:j1], wst[:],
                                     h1s[:, lo + j - doff : lo + j1 - doff],
                                     start=True, stop=True)
            h2 = h2p.tile([128, 1024], BF16, tag="h2",
                          name=f"h2_{s}_{g}_{aoff}")[:, 0:wsub]
            br_auto(h2, p, wt["t2"][:])
            return h2

        def unit_p3(sub, h2_ap, half):
            (s, g, gp, aoff, wsub, segs) = sub
            wst = wt["w3a"] if half == 0 else wt["w3b"]
            bias = wt["t3a"] if half == 0 else wt["t3b"]
            base = g * CHUNK + aoff
            acc = (accA if half == 0 else accB)[:, base : base + wsub]
            p = psum.tile([128, 1024], F32, tag="ps", space="PSUM",
                          name=f"p3{half}_{s}_{g}_{aoff}")[:, 0:wsub]
            for j, j1 in seg512(0, wsub):
                nc.tensor.matmul(p[:, j:j1], wst[:], h2_ap[:, j:j1],
                                 start=True, stop=True)
            if s == 0:
                br_auto(acc, p, bias[:])
            else:
                tt = tmp.tile([128, 1024], BF16, tag="tt",
                              name=f"tt{half}_{s}_{g}_{aoff}")[:, 0:wsub]
                br_auto(tt, p, bias[:])
                debt["dve"] += (wsub / 2.0 + 151.0) / 0.96 + 160.0
                nc.vector.tensor_max(acc, tt, acc)

        def unit_fin(g):
            for j in (0, 1024):
                p = psum.tile([128, 1024], F32, tag="ps", space="PSUM",
                              name=f"pc_{g}_{j}")
                for j2 in (0, 512):
                    for q in range(2):
                        cols = (g + q) * CHUNK + j + j2
                        dst = p[64 * q : 64 * q + 64, j2 : j2 + 512]
                        nc.tensor.matmul(dst, wt["wc0"][:],
                                         accA[:, cols : cols + 512],
                                         start=True, stop=False)
                        nc.tensor.matmul(dst, wt["wc1"][:],
                                         accB[:, cols : cols + 512],
                                         start=False, stop=True)
                sc = scp.tile([128, 1024], BF16, tag="sc", name=f"sc_{g}_{j}")
                br_auto(sc[:], p[:], wt["bc2"][:])
                for q in range(2):
                    uw = min(1024, w0.get(g + q, 0) - j)
                    if uw <= 0:
                        continue
                    o = (g + q) * CHUNK + j
                    nc.gpsimd.dma_start(out=comp[:, o : o + uw],
                                        in_=sc[64 * q : 64 * q + 64, 0:uw])

        # ---- software-pipelined emission over sub-items ----
        subs = _sub_items(items)
        last_sub = {}
        for i, sub in enumerate(subs):
            last_sub[sub[1]] = i  # last sub index per chunk
        done = [False] * G
        finned = [False] * G

        def try_fins(g):
            done[g] = True
            gp = g - (g % 2)
            if done[gp] and done[gp + 1] and not finned[gp]:
                finned[gp] = True
                unit_fin(gp)

        have = {sub[1] for sub in subs}
        for g in range(G):
            if g not in have:
                done[g] = True
        for g in range(0, G, 2):
            if done[g] and done[g + 1]:
                finned[g] = True

        p2q = list(enumerate(subs))
        p3q = []

        def pump(grp_done_upto):
            while p3q:
                i, sub, h2a = p3q.pop(0)
                unit_p3(sub, h2a, 0)
                unit_p3(sub, h2a, 1)
                if i == last_sub[sub[1]]:
                    try_fins(sub[1])
            while p2q and p2q[0][1][2] < grp_done_upto:
                i, sub = p2q.pop(0)
                p3q.append((i, sub, unit_p2(sub)))

        issue_vox(2)
        for t in range(G1):
            issue_vox(t + 3)
            unit_p1(t)
            pump(t)
        pump(G1)
        pump(G1)

    nc.compile()
    return nc


# ------------------------------------------------------------------ driver

_CACHE = {}


def kernel(voxels, coors, batch_size, w1, g1, b1, m1, v1,
           w2, g2, b2, m2, v2, w3, g3, b3, m3, v3, wc, bc,
           _trace=False):
    voxels = np.asarray(voxels, np.float32)
    coors = np.asarray(coors, np.int32)
    plan_key, items, G1, vox_all, rows_all = _build_plan_and_data(
        voxels, coors)
    G = plan_key[0]

    if plan_key not in _CACHE:
        _CACHE[plan_key] = build_program(G, items, G1)
    nc = _CACHE[plan_key]

    folded = _fold_weights(
        np.asarray(w1, np.float32), np.asarray(g1, np.float32),
        np.asarray(b1, np.float32), np.asarray(m1, np.float32),
        np.asarray(v1, np.float32),
        np.asarray(w2, np.float32), np.asarray(g2, np.float32),
        np.asarray(b2, np.float32), np.asarray(m2, np.float32),
        np.asarray(v2, np.float32),
        np.asarray(w3, np.float32), np.asarray(g3, np.float32),
        np.asarray(b3, np.float32), np.asarray(m3, np.float32),
        np.asarray(v3, np.float32),
        np.asarray(wc, np.float32), np.asarray(bc, np.float32))

    import ml_dtypes
    in_maps = [
        {"vox": vox_all[c].astype(ml_dtypes.bfloat16), **folded}
        for c in range(N_CORES)
    ]
    res = bass_utils.run_bass_kernel_spmd(
        nc, in_maps, core_ids=list(range(N_CORES)), trace=_trace)

    out = np.zeros((B, 64, GX * GY), np.float32)
    for c in range(N_CORES):
        cm = np.asarray(res.results[c]["comp"]).astype(np.float32)
        rows = rows_all[c]
        real = rows != PAD
        gcell = rows[real] + c * CELLS_PER_CORE
        b_core = c // (N_CORES // B)
        xy = gcell - b_core * (GX * GY)
        out[b_core][:, xy] = cm[:, real]
    out = out.reshape(B, 64, GX, GY)
    if _trace:
        return out, res
    return out
